# revision 24
# baseline (speedup 1.0000x reference)
"""CMXBlock (dense transformer block) Trainium2 Bass kernel.

Sharding: data-parallel over batch B=8 across the 8 NeuronCores — one image
per core, all weights replicated, no collectives.

Per-core computation (C=256 channels on partitions, HW=1024 positions free):
  x1 <- x1 + proj(softmax((q_w@bn1(x1))^T (k_w@bn1(x2)) * temp) @ (v_w@bn1(x2))^T)
  x1 <- x1 + fc2(gelu(dwconv3x3(fc1(bn2(x1)))))

Implementation notes (v4):
 - All matmul operands are 16-bit: fp16 for activations/weights, bf16 for
   the softmax numerators (range up to e^~30 overflows fp16).  16-bit
   stationary operands enable FWL; 16-bit moving operands let ACT/DVE feed
   the PE directly.
 - BatchNorms/temp folded into the 1x1-conv weights host-side.
 - Softmax: transposed scores S^T[m,n], 2 heads row-tiled (K=32 at row
   groups hb, hb+32), exp on ACT straight to bf16, denominator via an
   all-ones column in the AV stationary, both heads' AV col-tiled into one
   shared PSUM tile (ja at 0:33 / col groups 0-1, jb at 64:97 / 2-3).
 - The image is processed in two query/pixel halves: attention(half 1)'s
   ACT-bound softmax overlaps the MLP of half 0 on the PE (the MLP emission
   is a generator stepped from inside the attention loop).  During
   attention(half 0) zero-operand filler matmuls keep the PE HAM clock at
   K=8/8.
 - gelu is computed as x*0.5*(1+tanh(0.886x)) (max abs err 0.014): tanh
   lives in the same ACT table set as exp, so the interleaved softmax/MLP
   phases never thrash the 2.7us ACT table load.  The 0.5 and the (1+tanh)
   combine are folded into the fc2 weights / one DVE op.
 - Depthwise 3x3 runs on the PE as 9 diagonal-matrix taps over an x-padded
   [32, 36] spatial layout; all 72 diagonal tap tiles are preloaded in one
   DMA (per-tap DMA streaming was the MLP bottleneck).
 - MLP row split: fc1 rows 0-15 / 16-31 (aligns with proj's n-halves), dw
   and gelu rows 0-14 / 15-31 (one-row halo), fc2 columns 0-480 / 480-1024.
"""
import numpy as np

import concourse.bass as bass
import concourse.tile as tile
import concourse.mybir as mybir
from concourse import bacc
from concourse.bass_utils import run_bass_kernel_spmd

import concourse.bass_utils as _bu

if not getattr(_bu, "_ldwopt_patched", False):
    _orig_run_command = _bu.run_command

    def _run_command_ldwopt(cmd, **kw):
        cmd = list(cmd)  # ldw-opt=true fails walrus visitInstLdweights codegen
        return _orig_run_command(cmd, **kw)

    _bu.run_command = _run_command_ldwopt
    _bu._ldwopt_patched = True

F32 = mybir.dt.float32
BF16 = mybir.dt.bfloat16
FP16 = mybir.dt.float16
AF = mybir.ActivationFunctionType
ALU = mybir.AluOpType

B, C, H, W = 8, 256, 32, 32
NH, DH = 8, 32          # heads, head dim
HW = H * W              # 1024 positions
HID = 4 * C             # 1024 mlp hidden channels
EPS = 1e-5
WP = W + 4              # x-padded row width (36, even)
PADF = H * WP           # padded flat spatial size (1152)
N_CORES = 8
GA = 0.886              # tanh-gelu constant
NHALF = 512             # positions per query half

_NC_CACHE = {}


def _dw_chunks(shift, base, wl, wh, nrows):
    """Even-aligned bank chunks (<=512) of a dw tap's local dst range.

    Local dst f in [0, nrows*36); full-flat src = base + f + shift must lie
    in the written h1 window [wl, wh).  Elements dropped by even-alignment
    are x-pad columns (never read downstream) covered by the center tap.
    """
    n = nrows * WP
    lo = max(0, wl - base - shift)
    hi = min(n, wh - base - shift)
    out = []
    for b0 in range(0, n, 512):
        a, b = max(lo, b0), min(hi, b0 + 512)
        a += a % 2
        m = (b - a) & ~1
        if m > 0:
            out.append((a, m))
    return out


def _build_body(nc, tc, io):
    x1d, x2d = io["x1"], io["x2"]
    outd = io["out"]

    import contextlib
    ctx = contextlib.ExitStack()
    with ctx:
        wpool = ctx.enter_context(tc.tile_pool(name="weights", bufs=1))
        pB = ctx.enter_context(tc.tile_pool(name="pB", bufs=1))

        # ---------- persistent SBUF tensors ----------
        # DMA order matters: the first compute (q = qT @ x1) can start once
        # x1+qT land; the big late-use tensors (fc*, dwd) stream in last,
        # overlapped under attention.
        x1 = wpool.tile([128, 2, HW], FP16, tag="x1")
        nc.sync.dma_start(x1[:], x1d[:])

        def wload(name, shape, dt):
            t = wpool.tile(shape, dt, tag=name)
            nc.sync.dma_start(t[:], io[name][:])
            return t

        qT = wload("qT", [128, 2, C], FP16)
        qb = wload("qb", [128, 2], F32)
        x2 = wpool.tile([128, 2, HW], FP16, tag="x2")
        nc.sync.dma_start(x2[:], x2d[:])
        kT = wload("kT", [128, 2, C], FP16)
        kb = wload("kb", [128, 2], F32)
        inv1 = wload("inv1", [128, 2], F32)
        beta1 = wload("beta1", [128, 2], F32)
        vwT = wload("vwT", [128, 2, C], FP16)
        projT = wload("projT", [128, 2, C], FP16)
        projb = wload("projb", [128, 2], F32)
        fc1T = wload("fc1T", [128, 2, HID], FP16)
        fc1b = wload("fc1b", [128, 8], F32)
        fc2T = wload("fc2T", [128, 8, C], FP16)
        fc2b = wload("fc2b", [128, 2], F32)
        dwdt = wload("dwd", [128, 72, 128], FP16)   # 8ct x 9taps diag blocks

        attn_r = pB.tile([128, 2, HW], FP16, tag="attn_r")
        # PE-warmth filler operands (zeros; keep HAM at K=8/8 through the
        # ACT-bound softmax of query-half 0)
        dW = wpool.tile([128, 128], FP16, tag="dW")
        dM = wpool.tile([128, 512], FP16, tag="dM")
        sink = wpool.tile([1, 4], F32, tag="sink")
        nc.gpsimd.memset(dW[:], 0.0)
        nc.gpsimd.memset(dM[:], 0.0)

        ps2 = ctx.enter_context(tc.tile_pool(name="ps2", bufs=1, space="PSUM"))
        psd = ctx.enter_context(tc.tile_pool(name="psd", bufs=1, space="PSUM"))
        # filler accumulator borrows the (idle until mlp0) ps2 slot
        dum_ps = ps2.tile([128, 512], F32, tag="mm", name="dum")
        ndum = [0]

        def warm(n):
            for _ in range(n):
                nc.tensor.matmul(dum_ps[:], dW[:], dM[:],
                                 start=(ndum[0] == 0), stop=False,
                                 skip_group_check=True)
                ndum[0] += 1

        warm(10)   # keep the PE HAM clock spinning during the input DMAs

        pA = ctx.enter_context(tc.tile_pool(name="pA", bufs=1))
        q_sb = pA.tile([128, 2, HW], FP16, tag="q")
        k_sb = pA.tile([128, 2, HW], FP16, tag="k")
        vt1 = pA.tile([128, 8, NH, DH + 1], BF16, tag="vt1")  # [p, mt, h, v|1]
        onesf = pA.tile([128, 8 * NH], F32, tag="onesf")
        nc.gpsimd.memset(onesf[:], 1.0)
        nc.vector.tensor_copy(
            vt1[:, :, :, DH:DH + 1],
            onesf[:].rearrange("p (a b c) -> p a b c", a=8, b=NH))

        # ---------- phase 1: q, k projections; x2n; v^T ----------
        with tc.tile_pool(name="p1", bufs=1) as p1, \
             tc.tile_pool(name="ps1", bufs=4, space="PSUM") as ps1:
            for (wT, bias, dst) in ((qT, qb, q_sb), (kT, kb, k_sb)):
                rhs = x1 if dst is q_sb else x2
                for mt in range(2):
                    pss2 = [ps1.tile([128, 512], F32, tag="mm",
                                     name=f"qk_{mt}_{chk}") for chk in range(2)]
                    for kt in range(2):
                        for chk in range(2):
                            nc.tensor.matmul(
                                pss2[chk][:], wT[:, kt, 128 * mt:128 * (mt + 1)],
                                rhs[:, kt, 512 * chk:512 * (chk + 1)],
                                start=(kt == 0), stop=(kt == 1))
                    for chk in range(2):
                        nc.vector.tensor_scalar_add(
                            dst[:, mt, 512 * chk:512 * (chk + 1)], pss2[chk][:],
                            bias[:, mt:mt + 1])

            x2n = p1.tile([128, 2, HW], FP16, tag="x2n")
            for kt in range(2):
                nc.vector.tensor_scalar(
                    x2n[:, kt, :], x2[:, kt, :],
                    inv1[:, kt:kt + 1], beta1[:, kt:kt + 1], ALU.mult, ALU.add)

            for mp in range(8):
                ps = ps1.tile([128, 512], F32, tag="mm")
                for kt in range(2):
                    nc.tensor.matmul(
                        ps[:, 0:C], x2n[:, kt, 128 * mp:128 * (mp + 1)],
                        vwT[:, kt, :], start=(kt == 0), stop=(kt == 1))
                nc.vector.tensor_copy(
                    vt1[:, mp, :, 0:DH],
                    ps[:, 0:C].rearrange("p (h d) -> p h d", h=NH))

        # ---------- MLP state (written per query-half) ----------
        pC = ctx.enter_context(tc.tile_pool(name="pC", bufs=1))
        x1u = pC.tile([128, 2, HW], FP16, tag="x1u")
        h1 = pC.tile([128, 8, H, WP], FP16, tag="h1")
        zpad = pC.tile([128, 8 * H * 2], F32, tag="zpad")
        nc.gpsimd.memset(zpad[:], 0.0)
        zsrc = zpad[:].rearrange("p (c a b) -> p c a b", c=8, a=H)
        nc.vector.tensor_copy(h1[:, :, :, 0:2], zsrc)
        nc.vector.tensor_copy(h1[:, :, :, WP - 2:WP], zsrc)
        hgr = pC.tile([128, 8, HW], FP16, tag="hgr")
        out_sb = pC.tile([128, 2, HW], F32, tag="out")
        h1f = h1[:].rearrange("p c a b -> p c (a b)")
        taps = [(dy, dx) for dy in (-1, 0, 1) for dx in (-1, 0, 1)]
        taps.remove((0, 0))
        taps = [(0, 0)] + taps      # center first: full coverage, start=True

        def mlp_emit(nh, mmpool, dwpools):
            """Emit the MLP of query-half nh as ~1-2us PE units (generator).

            fc1 rows 16nh..16nh+15 (= proj's n-half exactly); dw/gelu out
            rows 0-14 / 15-31 (one-row halo into the other half's fc1
            output); fc2 columns 480*nh-split.
            """
            n0 = NHALF * nh
            # proj + residual1 (columns n0:n0+512)
            for mt in range(2):
                pp = mmpool.tile([128, 512], F32, tag="mm", name=f"pj{nh}_{mt}")
                for kt in range(2):
                    nc.tensor.matmul(
                        pp[:], projT[:, kt, 128 * mt:128 * (mt + 1)],
                        attn_r[:, kt, n0:n0 + 512],
                        start=(kt == 0), stop=(kt == 1))
                nc.vector.scalar_tensor_tensor(
                    x1u[:, mt, n0:n0 + 512], pp[:], projb[:, mt:mt + 1],
                    x1[:, mt, n0:n0 + 512], ALU.add, ALU.add)
                yield
            # fc1: rows 16nh..16nh+15 = columns n0:n0+512
            for mt in range(8):
                pf = mmpool.tile([128, 512], F32, tag="mm", name=f"f1{nh}_{mt}")
                for kt in range(2):
                    nc.tensor.matmul(
                        pf[:], fc1T[:, kt, 128 * mt:128 * (mt + 1)],
                        x1u[:, kt, n0:n0 + 512],
                        start=(kt == 0), stop=(kt == 1))
                nc.vector.tensor_scalar_add(
                    h1[:, mt, 16 * nh:16 * nh + 16, 2:W + 2],
                    pf[:].rearrange("p (a b) -> p a b", a=16),
                    fc1b[:, mt:mt + 1])
                if mt % 2 == 1:
                    yield
            # depthwise 3x3 + tanh-gelu, out rows r0..r1-1
            r0, r1 = (0, 15) if nh == 0 else (15, 32)
            nrows = r1 - r0
            base = r0 * WP
            wl = 0 if nh == 0 else 14 * WP
            wh = 16 * WP if nh == 0 else PADF
            g0, gn = 32 * r0, 32 * nrows
            for ct in range(8):
                # 1024 (2 exact banks) so 32-based partition slices stay
                # bank-aligned; only [0 : nrows*WP] is used
                ps_dw = dwpools[ct % len(dwpools)].tile(
                    [128, 1024], F32, tag="dw", name=f"dw{nh}_{ct}")
                for ti, (dy, dx) in enumerate(taps):
                    shift = dy * WP + dx
                    ws = 9 * ct + 3 * (dy + 1) + (dx + 1)
                    for (c0, n) in _dw_chunks(shift, base, wl, wh, nrows):
                        nc.tensor.matmul(
                            ps_dw[:, c0:c0 + n],
                            dwdt[:, ws, :],
                            h1f[:, ct, base + c0 + shift:base + c0 + shift + n],
                            start=(ti == 0), stop=(ti == len(taps) - 1))
                pin = ps_dw[:, 0:nrows * WP].rearrange(
                    "p (a b) -> p a b", a=nrows)[:, :, 2:W + 2]
                ut = pC.tile([128, 17 * W], F32, tag="ut")
                nc.scalar.activation(ut[:, 0:gn], pin, AF.Tanh, 0.0, GA)
                # hgr = (tanh + 1) * dw   (x0.5 folded into fc2 weights)
                nc.vector.scalar_tensor_tensor(
                    hgr[:, ct, g0:g0 + gn], ut[:, 0:gn], 1.0, pin,
                    ALU.add, ALU.mult)
                yield
            # fc2 + residual2 (columns g0:g0+gn)
            for mt in range(2):
                for cc0 in range(g0, g0 + gn, 512):
                    cn = min(512, g0 + gn - cc0)
                    pg = mmpool.tile([128, 512], F32, tag="mm",
                                  name=f"f2{nh}_{mt}_{cc0}")
                    for kt in range(8):
                        nc.tensor.matmul(
                            pg[:, 0:cn], fc2T[:, kt, 128 * mt:128 * (mt + 1)],
                            hgr[:, kt, cc0:cc0 + cn],
                            start=(kt == 0), stop=(kt == 7))
                    nc.vector.scalar_tensor_tensor(
                        out_sb[:, mt, cc0:cc0 + cn], pg[:, 0:cn],
                        fc2b[:, mt:mt + 1], x1u[:, mt, cc0:cc0 + cn],
                        ALU.add, ALU.add)
                    yield
            nc.sync.dma_start(outd[:, :, g0:g0 + gn], out_sb[:, :, g0:g0 + gn])

        # ---------- phase 2+3+4: attention halves with MLP overlap ----------
        filler = [None]
        dum_open = [True]

        def step():
            if filler[0] is not None:
                if next(filler[0], StopIteration) is StopIteration:
                    filler[0] = None
            elif dum_open[0]:
                warm(5)

        with tc.tile_pool(name="expS", bufs=6) as xpool, \
             tc.tile_pool(name="uz", bufs=3) as uz_pool, \
             tc.tile_pool(name="rtmp", bufs=4) as rpool, \
             tc.tile_pool(name="pss", bufs=2, space="PSUM") as pss, \
             tc.tile_pool(name="psa", bufs=1, space="PSUM") as psa:
            for nh in range(2):
                n0 = NHALF * nh
                for half in range(2):
                    for pr in range(2):
                        ja, jb = 2 * pr, 2 * pr + 1
                        heads = [(half * 4 + ja, 32 * ja, 0),
                                 (half * 4 + jb, 32 * jb, 64)]
                        ps_av = psa.tile([128, NHALF], F32, tag="av",
                                         name=f"av{nh}_{half}_{pr}")
                        pend = {h: None for h, _, _ in heads}
                        for mtp in range(4):
                            ps_s = {h: pss.tile([128, 2, NHALF], F32, tag="s",
                                                name=f"s{nh}_{h}_{mtp}")
                                    for h, _, _ in heads}
                            for h, hb, _ in heads:
                                for sub in range(2):
                                    mt = 2 * mtp + sub
                                    nc.tensor.matmul(
                                        ps_s[h][:, sub, :],
                                        k_sb[hb:hb + 32, half,
                                             128 * mt:128 * (mt + 1)],
                                        q_sb[hb:hb + 32, half, n0:n0 + NHALF],
                                        start=True, stop=True,
                                        tile_position=(hb, 0))
                            for h, hb, cb in heads:
                                ex = xpool.tile([128, 2, NHALF], BF16,
                                                tag="expS")
                                nc.scalar.activation(ex[:], ps_s[h][:], AF.Exp)
                                if pend[h] is not None:
                                    pmtp, pex = pend[h]
                                    for sub in range(2):
                                        nc.tensor.matmul(
                                            ps_av[cb:cb + DH + 1, :],
                                            vt1[:, 2 * pmtp + sub, h, :],
                                            pex[:, sub, :],
                                            start=(pmtp == 0 and sub == 0),
                                            stop=False,
                                            tile_position=(0, cb),
                                            skip_group_check=True)
                                pend[h] = (mtp, ex)
                            step()
                        for h, hb, cb in heads:
                            pmtp, pex = pend[h]
                            for sub in range(2):
                                nc.tensor.matmul(
                                    ps_av[cb:cb + DH + 1, :],
                                    vt1[:, 2 * pmtp + sub, h, :],
                                    pex[:, sub, :],
                                    start=False, stop=(sub == 1),
                                    tile_position=(0, cb),
                                    skip_group_check=True)
                        # evict + normalize per head (base-0 tiles: TT needs
                        # equal base partitions for both SBUF inputs)
                        for h, hb, cb in heads:
                            uz = uz_pool.tile([DH + 1, NHALF], F32, tag="uz")
                            nc.vector.tensor_copy(uz[:],
                                                  ps_av[cb:cb + DH + 1, :])
                            rt = rpool.tile([1, NHALF], F32, tag="rt")
                            nc.sync.dma_start(rt[:], uz[DH:DH + 1, :])
                            rr = rpool.tile([1, NHALF], F32, tag="rr")
                            nc.vector.reciprocal_approx_fast(rr[:], rt[:])
                            rb = rpool.tile([32, NHALF], F32, tag="rb")
                            nc.gpsimd.partition_broadcast(rb[:], rr[:])
                            nc.vector.tensor_mul(
                                attn_r[hb:hb + 32, half, n0:n0 + NHALF],
                                uz[0:DH, :], rb[:])
                        step()
                if nh == 0:
                    # close + consume the filler accumulation group (frees
                    # its PSUM bank for the nh=1 window: pss 4 + psa 1 +
                    # ps2 1 + psd 2 = 8 banks exactly)
                    nc.tensor.matmul(dum_ps[:], dW[:], dM[:],
                                     start=(ndum[0] == 0), stop=True,
                                     skip_group_check=True)
                    nc.vector.tensor_copy(sink[:], dum_ps[0:1, 0:4])
                    dum_open[0] = False
                    filler[0] = mlp_emit(0, ps2, [psd])
            while filler[0] is not None:
                step()

        with tc.tile_pool(name="mm2", bufs=3, space="PSUM") as ps2b, \
             tc.tile_pool(name="psd2", bufs=1, space="PSUM") as psd2:
            for _ in mlp_emit(1, ps2b, [psd, psd2]):
                pass


def _build_nc():
    if "nc" in _NC_CACHE:
        return _NC_CACHE["nc"]
    nc = bacc.Bacc(trn_type="TRN2", target_bir_lowering=False, debug=False)
    io = {}
    for name, shape, dt in [
        ("x1", [128, 2, HW], FP16), ("x2", [128, 2, HW], FP16),
        ("qT", [128, 2, C], FP16), ("kT", [128, 2, C], FP16),
        ("vwT", [128, 2, C], FP16), ("projT", [128, 2, C], FP16),
        ("fc1T", [128, 2, HID], FP16), ("fc2T", [128, 8, C], FP16),
        ("dwd", [128, 72, 128], FP16),
        ("qb", [128, 2], F32), ("kb", [128, 2], F32), ("projb", [128, 2], F32),
        ("fc1b", [128, 8], F32), ("fc2b", [128, 2], F32),
        ("inv1", [128, 2], F32), ("beta1", [128, 2], F32),
    ]:
        io[name] = nc.dram_tensor(name, shape, dt, kind="ExternalInput").ap()
    io["out"] = nc.dram_tensor("out", [128, 2, HW], F32, kind="ExternalOutput").ap()

    with tile.TileContext(nc) as tc:
        _build_body(nc, tc, io)
    nc.compile()
    _NC_CACHE["nc"] = nc
    return nc


def _to_part_layout(a, ntiles):
    """[ntiles*128, F] -> [128, ntiles, F] with c = kt*128 + p."""
    return np.ascontiguousarray(
        a.reshape(ntiles, 128, -1).transpose(1, 0, 2))


def _bias_layout(b, ntiles):
    """[ntiles*128] -> [128, ntiles]."""
    return np.ascontiguousarray(b.reshape(ntiles, 128).T)


def _prepare_weights(bn1_g, bn1_b, bn1_m, bn1_v, q_w, k_w, v_w, temp, proj_w,
                     proj_b, bn2_g, bn2_b, bn2_m, bn2_v, fc1_w, fc1_b, dw_w,
                     fc2_w, fc2_b):
    f64 = np.float64
    inv1 = (bn1_g.astype(f64) / np.sqrt(bn1_v.astype(f64) + EPS))
    beta1 = bn1_b.astype(f64) - bn1_m.astype(f64) * inv1
    inv2 = (bn2_g.astype(f64) / np.sqrt(bn2_v.astype(f64) + EPS))
    beta2 = bn2_b.astype(f64) - bn2_m.astype(f64) * inv2

    tscale = np.repeat(temp.astype(f64), DH)                     # [256]
    qw_f = q_w.astype(f64) * inv1[None, :] * tscale[:, None]
    qb = (q_w.astype(f64) @ beta1) * tscale
    kw_f = k_w.astype(f64) * inv1[None, :]
    kb = k_w.astype(f64) @ beta1
    fc1w_f = fc1_w.astype(f64) * inv2[None, :]
    fc1bf = fc1_b.astype(f64) + fc1_w.astype(f64) @ beta2

    # [128, 72, 128]: partition-major diag blocks, (ct, tap) on free dim
    dwd = np.zeros((128, 72, 128), np.float32)
    idx = np.arange(128)
    for ct in range(8):
        for t in range(9):
            dy, dx = t // 3, t % 3
            dwd[idx, ct * 9 + t, idx] = dw_w[ct * 128 + idx, 0, dy, dx]

    f16 = np.float16
    w = {
        "qT": _to_part_layout(np.ascontiguousarray(qw_f.T).astype(f16), 2),
        "kT": _to_part_layout(np.ascontiguousarray(kw_f.T).astype(f16), 2),
        "vwT": _to_part_layout(np.ascontiguousarray(v_w.T).astype(f16), 2),
        "projT": _to_part_layout(np.ascontiguousarray(proj_w.T).astype(f16), 2),
        "fc1T": _to_part_layout(np.ascontiguousarray(fc1w_f.T).astype(f16), 2),
        # 0.5 of the tanh-gelu form folded into fc2
        "fc2T": _to_part_layout(
            np.ascontiguousarray(0.5 * fc2_w.astype(f64).T).astype(f16), 8),
        "dwd": dwd.astype(f16),
        "qb": _bias_layout(qb.astype(np.float32), 2),
        "kb": _bias_layout(kb.astype(np.float32), 2),
        "projb": _bias_layout(proj_b.astype(np.float32), 2),
        "fc1b": _bias_layout(fc1bf.astype(np.float32), 8),
        "fc2b": _bias_layout(fc2_b.astype(np.float32), 2),
        "inv1": _bias_layout(inv1.astype(np.float32), 2),
        "beta1": _bias_layout(beta1.astype(np.float32), 2),
    }
    return w


_LAST_RESULTS = {}


def kernel(x1, x2, bn1_g, bn1_b, bn1_m, bn1_v, q_w, k_w, v_w, temp, proj_w,
           proj_b, bn2_g, bn2_b, bn2_m, bn2_v, fc1_w, fc1_b, dw_w, fc2_w,
           fc2_b, _trace=False):
    x1 = np.asarray(x1, np.float32)
    x2 = np.asarray(x2, np.float32)
    args = [np.asarray(a) for a in
            (bn1_g, bn1_b, bn1_m, bn1_v, q_w, k_w, v_w, temp, proj_w, proj_b,
             bn2_g, bn2_b, bn2_m, bn2_v, fc1_w, fc1_b, dw_w, fc2_w, fc2_b)]
    w = _prepare_weights(*args)

    nc = _build_nc()
    in_maps = []
    for i in range(N_CORES):
        m = dict(w)
        m["x1"] = _to_part_layout(x1[i].reshape(C, HW), 2).astype(np.float16)
        m["x2"] = _to_part_layout(x2[i].reshape(C, HW), 2).astype(np.float16)
        in_maps.append(m)

    res = run_bass_kernel_spmd(nc, in_maps, core_ids=list(range(N_CORES)),
                               trace=_trace)
    _LAST_RESULTS["res"] = res

    out = np.empty((B, C, H, W), np.float32)
    for i in range(N_CORES):
        o = res.results[i]["out"]                    # [128, 2, 1024]
        out[i] = o.transpose(1, 0, 2).reshape(C, H, W)
    return out


# revision 25
# speedup vs baseline: 1.1315x; 1.1315x over previous
"""CMXBlock (dense transformer block) Trainium2 Bass kernel.

Sharding: data-parallel over batch B=8 across the 8 NeuronCores — one image
per core, all weights replicated, no collectives.

Per-core computation (C=256 channels on partitions, HW=1024 positions free):
  x1 <- x1 + proj(softmax((q_w@bn1(x1))^T (k_w@bn1(x2)) * temp) @ (v_w@bn1(x2))^T)
  x1 <- x1 + fc2(gelu(dwconv3x3(fc1(bn2(x1)))))

Implementation notes (v4):
 - All matmul operands are 16-bit: fp16 for activations/weights, bf16 for
   the softmax numerators (range up to e^~30 overflows fp16).  16-bit
   stationary operands enable FWL; 16-bit moving operands let ACT/DVE feed
   the PE directly.
 - BatchNorms/temp folded into the 1x1-conv weights host-side.
 - Softmax: transposed scores S^T[m,n], 2 heads row-tiled (K=32 at row
   groups hb, hb+32), exp on ACT straight to bf16, denominator via an
   all-ones column in the AV stationary, both heads' AV col-tiled into one
   shared PSUM tile (ja at 0:33 / col groups 0-1, jb at 64:97 / 2-3).
 - The image is processed in two query/pixel halves: attention(half 1)'s
   ACT-bound softmax overlaps the MLP of half 0 on the PE (the MLP emission
   is a generator stepped from inside the attention loop).  During
   attention(half 0) zero-operand filler matmuls keep the PE HAM clock at
   K=8/8.
 - gelu is computed as x*0.5*(1+tanh(0.886x)) (max abs err 0.014): tanh
   lives in the same ACT table set as exp, so the interleaved softmax/MLP
   phases never thrash the 2.7us ACT table load.  The 0.5 and the (1+tanh)
   combine are folded into the fc2 weights / one DVE op.
 - Depthwise 3x3 runs on the PE as 9 diagonal-matrix taps over an x-padded
   [32, 36] spatial layout; all 72 diagonal tap tiles are preloaded in one
   DMA (per-tap DMA streaming was the MLP bottleneck).
 - MLP row split: fc1 rows 0-15 / 16-31 (aligns with proj's n-halves), dw
   and gelu rows 0-14 / 15-31 (one-row halo), fc2 columns 0-480 / 480-1024.
"""
import numpy as np

import concourse.bass as bass
import concourse.tile as tile
import concourse.mybir as mybir
from concourse import bacc
from concourse.bass_utils import run_bass_kernel_spmd

import concourse.bass_utils as _bu

if not getattr(_bu, "_ldwopt_patched", False):
    _orig_run_command = _bu.run_command

    def _run_command_ldwopt(cmd, **kw):
        cmd = list(cmd)  # ldw-opt=true fails walrus visitInstLdweights codegen
        return _orig_run_command(cmd, **kw)

    _bu.run_command = _run_command_ldwopt
    _bu._ldwopt_patched = True

F32 = mybir.dt.float32
BF16 = mybir.dt.bfloat16
FP16 = mybir.dt.float16
AF = mybir.ActivationFunctionType
ALU = mybir.AluOpType

B, C, H, W = 8, 256, 32, 32
NH, DH = 8, 32          # heads, head dim
HW = H * W              # 1024 positions
HID = 4 * C             # 1024 mlp hidden channels
EPS = 1e-5
WP = W + 4              # x-padded row width (36, even)
PADF = H * WP           # padded flat spatial size (1152)
N_CORES = 8
GA = 0.886              # tanh-gelu constant
NHALF = 512             # positions per query half

_NC_CACHE = {}


def _dw_chunks(shift, base, wl, wh, nrows):
    """Even-aligned bank chunks (<=512) of a dw tap's local dst range.

    Local dst f in [0, nrows*36); full-flat src = base + f + shift must lie
    in the written h1 window [wl, wh).  Elements dropped by even-alignment
    are x-pad columns (never read downstream) covered by the center tap.
    """
    n = nrows * WP
    lo = max(0, wl - base - shift)
    hi = min(n, wh - base - shift)
    out = []
    for b0 in range(0, n, 512):
        a, b = max(lo, b0), min(hi, b0 + 512)
        a += a % 2
        m = (b - a) & ~1
        if m > 0:
            out.append((a, m))
    return out


def _build_body(nc, tc, io):
    x1d, x2d = io["x1"], io["x2"]
    outd = io["out"]

    import contextlib
    ctx = contextlib.ExitStack()
    with ctx:
        wpool = ctx.enter_context(tc.tile_pool(name="weights", bufs=1))
        pB = ctx.enter_context(tc.tile_pool(name="pB", bufs=1))

        # ---------- persistent SBUF tensors ----------
        # DMA order matters: the first compute (q = qT @ x1) can start once
        # x1+qT land; the big late-use tensors (fc*, dwd) stream in last,
        # overlapped under attention.
        x1 = wpool.tile([128, 2, HW], FP16, tag="x1")
        nc.sync.dma_start(x1[:], x1d[:])

        def wload(name, shape, dt):
            t = wpool.tile(shape, dt, tag=name)
            nc.sync.dma_start(t[:], io[name][:])
            return t

        qT = wload("qT", [128, 2, C], FP16)
        qb = wload("qb", [128, 2], F32)
        x2 = wpool.tile([128, 2, HW], FP16, tag="x2")
        nc.sync.dma_start(x2[:], x2d[:])
        kT = wload("kT", [128, 2, C], FP16)
        kb = wload("kb", [128, 2], F32)
        inv1 = wload("inv1", [128, 2], F32)
        beta1 = wload("beta1", [128, 2], F32)
        vwT = wload("vwT", [128, 2, C], FP16)
        projT = wload("projT", [128, 2, C], FP16)
        projb = wload("projb", [128, 2], F32)
        fc1T = wload("fc1T", [128, 2, HID], FP16)
        fc1b = wload("fc1b", [128, 8], F32)
        fc2T = wload("fc2T", [128, 8, C], FP16)
        fc2b = wload("fc2b", [128, 2], F32)
        dwdt = wload("dwd", [128, 72, 128], FP16)   # 8ct x 9taps diag blocks

        attn_r = pB.tile([128, 2, HW], FP16, tag="attn_r")
        # PE-warmth filler operands (zeros; keep HAM at K=8/8 through the
        # ACT-bound softmax of query-half 0)
        dW = wpool.tile([128, 128], FP16, tag="dW")
        dM = wpool.tile([128, 512], FP16, tag="dM")
        sink = wpool.tile([1, 4], F32, tag="sink")
        nc.gpsimd.memset(dW[:], 0.0)
        nc.gpsimd.memset(dM[:], 0.0)

        ps2 = ctx.enter_context(tc.tile_pool(name="ps2", bufs=1, space="PSUM"))
        psd = ctx.enter_context(tc.tile_pool(name="psd", bufs=1, space="PSUM"))
        # filler accumulator borrows the (idle until mlp0) ps2 slot
        dum_ps = ps2.tile([128, 512], F32, tag="mm", name="dum")
        ndum = [0]

        def warm(n):
            for _ in range(n):
                nc.tensor.matmul(dum_ps[:], dW[:], dM[:],
                                 start=(ndum[0] == 0), stop=False,
                                 skip_group_check=True)
                ndum[0] += 1

        warm(16)   # keep the PE HAM clock spinning during the input DMAs

        pA = ctx.enter_context(tc.tile_pool(name="pA", bufs=1))
        q_sb = pA.tile([128, 2, HW], FP16, tag="q")
        k_sb = pA.tile([128, 2, HW], FP16, tag="k")
        vt1 = pA.tile([128, 8, NH, DH + 1], BF16, tag="vt1")  # [p, mt, h, v|1]
        onesf = pA.tile([128, 8 * NH], F32, tag="onesf")
        nc.gpsimd.memset(onesf[:], 1.0)
        nc.vector.tensor_copy(
            vt1[:, :, :, DH:DH + 1],
            onesf[:].rearrange("p (a b c) -> p a b c", a=8, b=NH))

        # ---------- phase 1: q, k projections; x2n; v^T ----------
        with tc.tile_pool(name="p1", bufs=1) as p1, \
             tc.tile_pool(name="ps1", bufs=4, space="PSUM") as ps1:
            for (wT, bias, dst) in ((qT, qb, q_sb), (kT, kb, k_sb)):
                rhs = x1 if dst is q_sb else x2
                for mt in range(2):
                    pss2 = [ps1.tile([128, 512], F32, tag="mm",
                                     name=f"qk_{mt}_{chk}") for chk in range(2)]
                    for kt in range(2):
                        for chk in range(2):
                            nc.tensor.matmul(
                                pss2[chk][:], wT[:, kt, 128 * mt:128 * (mt + 1)],
                                rhs[:, kt, 512 * chk:512 * (chk + 1)],
                                start=(kt == 0), stop=(kt == 1))
                    for chk in range(2):
                        nc.vector.tensor_scalar_add(
                            dst[:, mt, 512 * chk:512 * (chk + 1)], pss2[chk][:],
                            bias[:, mt:mt + 1])

            x2n = p1.tile([128, 2, HW], FP16, tag="x2n")
            for kt in range(2):
                nc.vector.tensor_scalar(
                    x2n[:, kt, :], x2[:, kt, :],
                    inv1[:, kt:kt + 1], beta1[:, kt:kt + 1], ALU.mult, ALU.add)

            for mp in range(8):
                ps = ps1.tile([128, 512], F32, tag="mm")
                for kt in range(2):
                    nc.tensor.matmul(
                        ps[:, 0:C], x2n[:, kt, 128 * mp:128 * (mp + 1)],
                        vwT[:, kt, :], start=(kt == 0), stop=(kt == 1))
                nc.vector.tensor_copy(
                    vt1[:, mp, :, 0:DH],
                    ps[:, 0:C].rearrange("p (h d) -> p h d", h=NH))

        # ---------- MLP state (written per query-half) ----------
        pC = ctx.enter_context(tc.tile_pool(name="pC", bufs=1))
        x1u = pC.tile([128, 2, HW], FP16, tag="x1u")
        h1 = pC.tile([128, 8, H, WP], FP16, tag="h1")
        zpad = pC.tile([128, 8 * H * 2], F32, tag="zpad")
        nc.gpsimd.memset(zpad[:], 0.0)
        zsrc = zpad[:].rearrange("p (c a b) -> p c a b", c=8, a=H)
        nc.vector.tensor_copy(h1[:, :, :, 0:2], zsrc)
        nc.vector.tensor_copy(h1[:, :, :, WP - 2:WP], zsrc)
        hgr = pC.tile([128, 8, HW], FP16, tag="hgr")
        out_sb = pC.tile([128, 2, HW], F32, tag="out")
        h1f = h1[:].rearrange("p c a b -> p c (a b)")
        taps = [(dy, dx) for dy in (-1, 0, 1) for dx in (-1, 0, 1)]
        taps.remove((0, 0))
        taps = [(0, 0)] + taps      # center first: full coverage, start=True

        def mlp_emit(nh, mmpool, dwpools):
            """Emit the MLP of query-half nh as ~1-2us PE units (generator).

            fc1 rows 16nh..16nh+15 (= proj's n-half exactly); dw/gelu out
            rows 0-14 / 15-31 (one-row halo into the other half's fc1
            output); fc2 columns 480*nh-split.
            """
            n0 = NHALF * nh
            # proj + residual1 (columns n0:n0+512)
            for mt in range(2):
                pp = mmpool.tile([128, 512], F32, tag="mm", name=f"pj{nh}_{mt}")
                for kt in range(2):
                    nc.tensor.matmul(
                        pp[:], projT[:, kt, 128 * mt:128 * (mt + 1)],
                        attn_r[:, kt, n0:n0 + 512],
                        start=(kt == 0), stop=(kt == 1))
                nc.vector.scalar_tensor_tensor(
                    x1u[:, mt, n0:n0 + 512], pp[:], projb[:, mt:mt + 1],
                    x1[:, mt, n0:n0 + 512], ALU.add, ALU.add)
                yield
            # fc1: rows 16nh..16nh+15 = columns n0:n0+512
            for mt in range(8):
                pf = mmpool.tile([128, 512], F32, tag="mm", name=f"f1{nh}_{mt}")
                for kt in range(2):
                    nc.tensor.matmul(
                        pf[:], fc1T[:, kt, 128 * mt:128 * (mt + 1)],
                        x1u[:, kt, n0:n0 + 512],
                        start=(kt == 0), stop=(kt == 1))
                nc.vector.tensor_scalar_add(
                    h1[:, mt, 16 * nh:16 * nh + 16, 2:W + 2],
                    pf[:].rearrange("p (a b) -> p a b", a=16),
                    fc1b[:, mt:mt + 1])
                if mt % 2 == 1:
                    yield
            # depthwise 3x3 + tanh-gelu, out rows r0..r1-1
            r0, r1 = (0, 15) if nh == 0 else (15, 32)
            nrows = r1 - r0
            base = r0 * WP
            wl = 0 if nh == 0 else 14 * WP
            wh = 16 * WP if nh == 0 else PADF
            g0, gn = 32 * r0, 32 * nrows
            for ct in range(8):
                ps_dw = dwpools[ct % len(dwpools)].tile(
                    [128, 17 * WP], F32, tag="dw", name=f"dw{nh}_{ct}")
                for ti, (dy, dx) in enumerate(taps):
                    shift = dy * WP + dx
                    ws = 9 * ct + 3 * (dy + 1) + (dx + 1)
                    for (c0, n) in _dw_chunks(shift, base, wl, wh, nrows):
                        nc.tensor.matmul(
                            ps_dw[:, c0:c0 + n],
                            dwdt[:, ws, :],
                            h1f[:, ct, base + c0 + shift:base + c0 + shift + n],
                            start=(ti == 0), stop=(ti == len(taps) - 1))
                pin = ps_dw[:, 0:nrows * WP].rearrange(
                    "p (a b) -> p a b", a=nrows)[:, :, 2:W + 2]
                ut = pC.tile([128, 17 * W], F32, tag="ut")
                nc.scalar.activation(ut[:, 0:gn], pin, AF.Tanh, 0.0, GA)
                # hgr = (tanh + 1) * dw   (x0.5 folded into fc2 weights)
                nc.vector.scalar_tensor_tensor(
                    hgr[:, ct, g0:g0 + gn], ut[:, 0:gn], 1.0, pin,
                    ALU.add, ALU.mult)
                yield
            # fc2 + residual2 (columns g0:g0+gn)
            for mt in range(2):
                for cc0 in range(g0, g0 + gn, 512):
                    cn = min(512, g0 + gn - cc0)
                    pg = mmpool.tile([128, 512], F32, tag="mm",
                                  name=f"f2{nh}_{mt}_{cc0}")
                    for kt in range(8):
                        nc.tensor.matmul(
                            pg[:, 0:cn], fc2T[:, kt, 128 * mt:128 * (mt + 1)],
                            hgr[:, kt, cc0:cc0 + cn],
                            start=(kt == 0), stop=(kt == 7))
                    nc.vector.scalar_tensor_tensor(
                        out_sb[:, mt, cc0:cc0 + cn], pg[:, 0:cn],
                        fc2b[:, mt:mt + 1], x1u[:, mt, cc0:cc0 + cn],
                        ALU.add, ALU.add)
                    yield
            nc.sync.dma_start(outd[:, :, g0:g0 + gn], out_sb[:, :, g0:g0 + gn])

        # ---------- phase 2+3+4: attention halves with MLP overlap ----------
        filler = [None]
        dum_open = [True]

        def step():
            if filler[0] is not None:
                if next(filler[0], StopIteration) is StopIteration:
                    filler[0] = None
            elif dum_open[0]:
                warm(5)

        with tc.tile_pool(name="expS", bufs=6) as xpool, \
             tc.tile_pool(name="uz", bufs=3) as uz_pool, \
             tc.tile_pool(name="rtmp", bufs=4) as rpool, \
             tc.tile_pool(name="pss", bufs=2, space="PSUM") as pss, \
             tc.tile_pool(name="psa", bufs=1, space="PSUM") as psa:
            for nh in range(2):
                n0 = NHALF * nh
                for half in range(2):
                    for pr in range(2):
                        ja, jb = 2 * pr, 2 * pr + 1
                        heads = [(half * 4 + ja, 32 * ja, 0),
                                 (half * 4 + jb, 32 * jb, 64)]
                        ps_av = psa.tile([128, NHALF], F32, tag="av",
                                         name=f"av{nh}_{half}_{pr}")
                        pend = {h: None for h, _, _ in heads}
                        for mtp in range(4):
                            ps_s = {h: pss.tile([128, 2, NHALF], F32, tag="s",
                                                name=f"s{nh}_{h}_{mtp}")
                                    for h, _, _ in heads}
                            for h, hb, _ in heads:
                                for sub in range(2):
                                    mt = 2 * mtp + sub
                                    nc.tensor.matmul(
                                        ps_s[h][:, sub, :],
                                        k_sb[hb:hb + 32, half,
                                             128 * mt:128 * (mt + 1)],
                                        q_sb[hb:hb + 32, half, n0:n0 + NHALF],
                                        start=True, stop=True,
                                        tile_position=(hb, 0))
                            for h, hb, cb in heads:
                                ex = xpool.tile([128, 2, NHALF], BF16,
                                                tag="expS")
                                nc.scalar.activation(ex[:], ps_s[h][:], AF.Exp)
                                if pend[h] is not None:
                                    pmtp, pex = pend[h]
                                    for sub in range(2):
                                        nc.tensor.matmul(
                                            ps_av[cb:cb + DH + 1, :],
                                            vt1[:, 2 * pmtp + sub, h, :],
                                            pex[:, sub, :],
                                            start=(pmtp == 0 and sub == 0),
                                            stop=False,
                                            tile_position=(0, cb),
                                            skip_group_check=True)
                                pend[h] = (mtp, ex)
                            step()
                        for h, hb, cb in heads:
                            pmtp, pex = pend[h]
                            for sub in range(2):
                                nc.tensor.matmul(
                                    ps_av[cb:cb + DH + 1, :],
                                    vt1[:, 2 * pmtp + sub, h, :],
                                    pex[:, sub, :],
                                    start=False, stop=(sub == 1),
                                    tile_position=(0, cb),
                                    skip_group_check=True)
                        # evict + normalize per head (base-0 tiles: TT needs
                        # equal base partitions for both SBUF inputs)
                        for h, hb, cb in heads:
                            uz = uz_pool.tile([DH + 1, NHALF], F32, tag="uz")
                            nc.vector.tensor_copy(uz[:],
                                                  ps_av[cb:cb + DH + 1, :])
                            rt = rpool.tile([1, NHALF], F32, tag="rt")
                            nc.sync.dma_start(rt[:], uz[DH:DH + 1, :])
                            rr = rpool.tile([1, NHALF], F32, tag="rr")
                            nc.vector.reciprocal_approx_fast(rr[:], rt[:])
                            rb = rpool.tile([32, NHALF], F32, tag="rb")
                            nc.gpsimd.partition_broadcast(rb[:], rr[:])
                            nc.vector.tensor_mul(
                                attn_r[hb:hb + 32, half, n0:n0 + NHALF],
                                uz[0:DH, :], rb[:])
                        step()
                if nh == 0:
                    # close + consume the filler accumulation group (frees
                    # its PSUM bank for the nh=1 window: pss 4 + psa 1 +
                    # ps2 1 + psd 2 = 8 banks exactly)
                    nc.tensor.matmul(dum_ps[:], dW[:], dM[:],
                                     start=(ndum[0] == 0), stop=True,
                                     skip_group_check=True)
                    nc.vector.tensor_copy(sink[:], dum_ps[0:1, 0:4])
                    dum_open[0] = False
                    filler[0] = mlp_emit(0, ps2, [psd])
            while filler[0] is not None:
                step()

        with tc.tile_pool(name="mm2", bufs=3, space="PSUM") as ps2b, \
             tc.tile_pool(name="psd2", bufs=1, space="PSUM") as psd2:
            for _ in mlp_emit(1, ps2b, [psd, psd2]):
                pass


def _build_nc():
    if "nc" in _NC_CACHE:
        return _NC_CACHE["nc"]
    nc = bacc.Bacc(trn_type="TRN2", target_bir_lowering=False, debug=False)
    io = {}
    for name, shape, dt in [
        ("x1", [128, 2, HW], FP16), ("x2", [128, 2, HW], FP16),
        ("qT", [128, 2, C], FP16), ("kT", [128, 2, C], FP16),
        ("vwT", [128, 2, C], FP16), ("projT", [128, 2, C], FP16),
        ("fc1T", [128, 2, HID], FP16), ("fc2T", [128, 8, C], FP16),
        ("dwd", [128, 72, 128], FP16),
        ("qb", [128, 2], F32), ("kb", [128, 2], F32), ("projb", [128, 2], F32),
        ("fc1b", [128, 8], F32), ("fc2b", [128, 2], F32),
        ("inv1", [128, 2], F32), ("beta1", [128, 2], F32),
    ]:
        io[name] = nc.dram_tensor(name, shape, dt, kind="ExternalInput").ap()
    io["out"] = nc.dram_tensor("out", [128, 2, HW], F32, kind="ExternalOutput").ap()

    with tile.TileContext(nc) as tc:
        _build_body(nc, tc, io)
    nc.compile()
    _NC_CACHE["nc"] = nc
    return nc


def _to_part_layout(a, ntiles):
    """[ntiles*128, F] -> [128, ntiles, F] with c = kt*128 + p."""
    return np.ascontiguousarray(
        a.reshape(ntiles, 128, -1).transpose(1, 0, 2))


def _bias_layout(b, ntiles):
    """[ntiles*128] -> [128, ntiles]."""
    return np.ascontiguousarray(b.reshape(ntiles, 128).T)


def _prepare_weights(bn1_g, bn1_b, bn1_m, bn1_v, q_w, k_w, v_w, temp, proj_w,
                     proj_b, bn2_g, bn2_b, bn2_m, bn2_v, fc1_w, fc1_b, dw_w,
                     fc2_w, fc2_b):
    f64 = np.float64
    inv1 = (bn1_g.astype(f64) / np.sqrt(bn1_v.astype(f64) + EPS))
    beta1 = bn1_b.astype(f64) - bn1_m.astype(f64) * inv1
    inv2 = (bn2_g.astype(f64) / np.sqrt(bn2_v.astype(f64) + EPS))
    beta2 = bn2_b.astype(f64) - bn2_m.astype(f64) * inv2

    tscale = np.repeat(temp.astype(f64), DH)                     # [256]
    qw_f = q_w.astype(f64) * inv1[None, :] * tscale[:, None]
    qb = (q_w.astype(f64) @ beta1) * tscale
    kw_f = k_w.astype(f64) * inv1[None, :]
    kb = k_w.astype(f64) @ beta1
    fc1w_f = fc1_w.astype(f64) * inv2[None, :]
    fc1bf = fc1_b.astype(f64) + fc1_w.astype(f64) @ beta2

    # [128, 72, 128]: partition-major diag blocks, (ct, tap) on free dim
    dwd = np.zeros((128, 72, 128), np.float32)
    idx = np.arange(128)
    for ct in range(8):
        for t in range(9):
            dy, dx = t // 3, t % 3
            dwd[idx, ct * 9 + t, idx] = dw_w[ct * 128 + idx, 0, dy, dx]

    f16 = np.float16
    w = {
        "qT": _to_part_layout(np.ascontiguousarray(qw_f.T).astype(f16), 2),
        "kT": _to_part_layout(np.ascontiguousarray(kw_f.T).astype(f16), 2),
        "vwT": _to_part_layout(np.ascontiguousarray(v_w.T).astype(f16), 2),
        "projT": _to_part_layout(np.ascontiguousarray(proj_w.T).astype(f16), 2),
        "fc1T": _to_part_layout(np.ascontiguousarray(fc1w_f.T).astype(f16), 2),
        # 0.5 of the tanh-gelu form folded into fc2
        "fc2T": _to_part_layout(
            np.ascontiguousarray(0.5 * fc2_w.astype(f64).T).astype(f16), 8),
        "dwd": dwd.astype(f16),
        "qb": _bias_layout(qb.astype(np.float32), 2),
        "kb": _bias_layout(kb.astype(np.float32), 2),
        "projb": _bias_layout(proj_b.astype(np.float32), 2),
        "fc1b": _bias_layout(fc1bf.astype(np.float32), 8),
        "fc2b": _bias_layout(fc2_b.astype(np.float32), 2),
        "inv1": _bias_layout(inv1.astype(np.float32), 2),
        "beta1": _bias_layout(beta1.astype(np.float32), 2),
    }
    return w


_LAST_RESULTS = {}


def kernel(x1, x2, bn1_g, bn1_b, bn1_m, bn1_v, q_w, k_w, v_w, temp, proj_w,
           proj_b, bn2_g, bn2_b, bn2_m, bn2_v, fc1_w, fc1_b, dw_w, fc2_w,
           fc2_b, _trace=False):
    x1 = np.asarray(x1, np.float32)
    x2 = np.asarray(x2, np.float32)
    args = [np.asarray(a) for a in
            (bn1_g, bn1_b, bn1_m, bn1_v, q_w, k_w, v_w, temp, proj_w, proj_b,
             bn2_g, bn2_b, bn2_m, bn2_v, fc1_w, fc1_b, dw_w, fc2_w, fc2_b)]
    w = _prepare_weights(*args)

    nc = _build_nc()
    in_maps = []
    for i in range(N_CORES):
        m = dict(w)
        m["x1"] = _to_part_layout(x1[i].reshape(C, HW), 2).astype(np.float16)
        m["x2"] = _to_part_layout(x2[i].reshape(C, HW), 2).astype(np.float16)
        in_maps.append(m)

    res = run_bass_kernel_spmd(nc, in_maps, core_ids=list(range(N_CORES)),
                               trace=_trace)
    _LAST_RESULTS["res"] = res

    out = np.empty((B, C, H, W), np.float32)
    for i in range(N_CORES):
        o = res.results[i]["out"]                    # [128, 2, 1024]
        out[i] = o.transpose(1, 0, 2).reshape(C, H, W)
    return out


# revision 26
# speedup vs baseline: 1.1926x; 1.0540x over previous
"""CMXBlock (dense transformer block) Trainium2 Bass kernel.

Sharding: data-parallel over batch B=8 across the 8 NeuronCores — one image
per core, all weights replicated, no collectives.

Per-core computation (C=256 channels on partitions, HW=1024 positions free):
  x1 <- x1 + proj(softmax((q_w@bn1(x1))^T (k_w@bn1(x2)) * temp) @ (v_w@bn1(x2))^T)
  x1 <- x1 + fc2(gelu(dwconv3x3(fc1(bn2(x1)))))

Implementation notes (v4):
 - All matmul operands are 16-bit: fp16 for activations/weights, bf16 for
   the softmax numerators (range up to e^~30 overflows fp16).  16-bit
   stationary operands enable FWL; 16-bit moving operands let ACT/DVE feed
   the PE directly.
 - BatchNorms/temp folded into the 1x1-conv weights host-side.
 - Softmax: transposed scores S^T[m,n], 2 heads row-tiled (K=32 at row
   groups hb, hb+32), exp on ACT straight to bf16, denominator via an
   all-ones column in the AV stationary, both heads' AV col-tiled into one
   shared PSUM tile (ja at 0:33 / col groups 0-1, jb at 64:97 / 2-3).
 - The image is processed in two query/pixel halves: attention(half 1)'s
   ACT-bound softmax overlaps the MLP of half 0 on the PE (the MLP emission
   is a generator stepped from inside the attention loop).  During
   attention(half 0) zero-operand filler matmuls keep the PE HAM clock at
   K=8/8.
 - gelu is computed as x*0.5*(1+tanh(0.886x)) (max abs err 0.014): tanh
   lives in the same ACT table set as exp, so the interleaved softmax/MLP
   phases never thrash the 2.7us ACT table load.  The 0.5 and the (1+tanh)
   combine are folded into the fc2 weights / one DVE op.
 - Depthwise 3x3 runs on the PE as 9 diagonal-matrix taps over an x-padded
   [32, 36] spatial layout; all 72 diagonal tap tiles are preloaded in one
   DMA (per-tap DMA streaming was the MLP bottleneck).
 - MLP row split: fc1 rows 0-15 / 16-31 (aligns with proj's n-halves), dw
   and gelu rows 0-14 / 15-31 (one-row halo), fc2 columns 0-480 / 480-1024.
"""
import numpy as np

import concourse.bass as bass
import concourse.tile as tile
import concourse.mybir as mybir
from concourse import bacc
from concourse.bass_utils import run_bass_kernel_spmd

import concourse.bass_utils as _bu

if not getattr(_bu, "_ldwopt_patched", False):
    _orig_run_command = _bu.run_command

    def _run_command_ldwopt(cmd, **kw):
        cmd = list(cmd)  # ldw-opt=true fails walrus visitInstLdweights codegen
        return _orig_run_command(cmd, **kw)

    _bu.run_command = _run_command_ldwopt
    _bu._ldwopt_patched = True

F32 = mybir.dt.float32
BF16 = mybir.dt.bfloat16
FP16 = mybir.dt.float16
AF = mybir.ActivationFunctionType
ALU = mybir.AluOpType

B, C, H, W = 8, 256, 32, 32
NH, DH = 8, 32          # heads, head dim
HW = H * W              # 1024 positions
HID = 4 * C             # 1024 mlp hidden channels
EPS = 1e-5
WP = W + 4              # x-padded row width (36, even)
PADF = H * WP           # padded flat spatial size (1152)
N_CORES = 8
GA = 0.886              # tanh-gelu constant
NHALF = 512             # positions per query half

_NC_CACHE = {}


def _dw_chunks(shift, base, wl, wh, nrows):
    """Even-aligned bank chunks (<=512) of a dw tap's local dst range.

    Local dst f in [0, nrows*36); full-flat src = base + f + shift must lie
    in the written h1 window [wl, wh).  Elements dropped by even-alignment
    are x-pad columns (never read downstream) covered by the center tap.
    """
    n = nrows * WP
    lo = max(0, wl - base - shift)
    hi = min(n, wh - base - shift)
    out = []
    for b0 in range(0, n, 512):
        a, b = max(lo, b0), min(hi, b0 + 512)
        a += a % 2
        m = (b - a) & ~1
        if m > 0:
            out.append((a, m))
    return out


def _build_body(nc, tc, io):
    x1d, x2d = io["x1"], io["x2"]
    outd = io["out"]

    import contextlib
    ctx = contextlib.ExitStack()
    with ctx:
        wpool = ctx.enter_context(tc.tile_pool(name="weights", bufs=1))
        pB = ctx.enter_context(tc.tile_pool(name="pB", bufs=1))

        # ---------- persistent SBUF tensors ----------
        # DMA order matters: the first compute (q = qT @ x1) can start once
        # x1+qT land; the big late-use tensors (fc*, dwd) stream in last,
        # overlapped under attention.
        x1 = wpool.tile([128, 2, HW], FP16, tag="x1")
        nc.sync.dma_start(x1[:], x1d[:])

        def wload(name, shape, dt):
            t = wpool.tile(shape, dt, tag=name)
            nc.sync.dma_start(t[:], io[name][:])
            return t

        qT = wload("qT", [128, 2, C], FP16)
        qb = wload("qb", [128, 2], F32)
        x2 = wpool.tile([128, 2, HW], FP16, tag="x2")
        nc.sync.dma_start(x2[:], x2d[:])
        kT = wload("kT", [128, 2, C], FP16)
        kb = wload("kb", [128, 2], F32)
        inv1 = wload("inv1", [128, 2], F32)
        beta1 = wload("beta1", [128, 2], F32)
        vwT = wload("vwT", [128, 2, C], FP16)
        projT = wload("projT", [128, 2, C], FP16)
        projb = wload("projb", [128, 2], F32)
        fc1T = wload("fc1T", [128, 2, HID], FP16)
        fc1b = wload("fc1b", [128, 8], F32)
        fc2T = wload("fc2T", [128, 8, C], FP16)
        fc2b = wload("fc2b", [128, 2], F32)
        dwdt = wload("dwd", [128, 72, 128], FP16)   # 8ct x 9taps diag blocks

        attn_r = pB.tile([128, 2, HW], FP16, tag="attn_r")
        # PE-warmth filler operands (zeros; keep HAM at K=8/8 through the
        # ACT-bound softmax of query-half 0)
        dW = wpool.tile([128, 128], FP16, tag="dW")
        dM = wpool.tile([128, 512], FP16, tag="dM")
        sink = wpool.tile([1, 4], F32, tag="sink")
        nc.gpsimd.memset(dW[:], 0.0)
        nc.gpsimd.memset(dM[:], 0.0)

        ps2 = ctx.enter_context(tc.tile_pool(name="ps2", bufs=1, space="PSUM"))
        psd = ctx.enter_context(tc.tile_pool(name="psd", bufs=1, space="PSUM"))
        # filler accumulator borrows the (idle until mlp0) ps2 slot
        dum_ps = ps2.tile([128, 512], F32, tag="mm", name="dum")
        ndum = [0]

        def warm(n):
            for _ in range(n):
                nc.tensor.matmul(dum_ps[:], dW[:], dM[:],
                                 start=(ndum[0] == 0), stop=False,
                                 skip_group_check=True)
                ndum[0] += 1

        warm(16)   # keep the PE HAM clock spinning during the input DMAs

        pA = ctx.enter_context(tc.tile_pool(name="pA", bufs=1))
        q_sb = pA.tile([128, 2, HW], FP16, tag="q")
        k_sb = pA.tile([128, 2, HW], FP16, tag="k")
        vt1 = pA.tile([128, 8, NH, DH + 1], BF16, tag="vt1")  # [p, mt, h, v|1]
        onesf = pA.tile([128, 8 * NH], F32, tag="onesf")
        nc.gpsimd.memset(onesf[:], 1.0)
        nc.vector.tensor_copy(
            vt1[:, :, :, DH:DH + 1],
            onesf[:].rearrange("p (a b c) -> p a b c", a=8, b=NH))

        # ---------- phase 1: q, k projections; x2n; v^T ----------
        with tc.tile_pool(name="p1", bufs=1) as p1, \
             tc.tile_pool(name="ps1", bufs=4, space="PSUM") as ps1:
            for (wT, bias, dst) in ((qT, qb, q_sb), (kT, kb, k_sb)):
                rhs = x1 if dst is q_sb else x2
                for mt in range(2):
                    pss2 = [ps1.tile([128, 512], F32, tag="mm",
                                     name=f"qk_{mt}_{chk}") for chk in range(2)]
                    for kt in range(2):
                        for chk in range(2):
                            nc.tensor.matmul(
                                pss2[chk][:], wT[:, kt, 128 * mt:128 * (mt + 1)],
                                rhs[:, kt, 512 * chk:512 * (chk + 1)],
                                start=(kt == 0), stop=(kt == 1))
                    for chk in range(2):
                        nc.vector.tensor_scalar_add(
                            dst[:, mt, 512 * chk:512 * (chk + 1)], pss2[chk][:],
                            bias[:, mt:mt + 1])

            x2n = p1.tile([128, 2, HW], FP16, tag="x2n")
            for kt in range(2):
                nc.vector.tensor_scalar(
                    x2n[:, kt, :], x2[:, kt, :],
                    inv1[:, kt:kt + 1], beta1[:, kt:kt + 1], ALU.mult, ALU.add)

            for mp in range(8):
                ps = ps1.tile([128, 512], F32, tag="mm")
                for kt in range(2):
                    nc.tensor.matmul(
                        ps[:, 0:C], x2n[:, kt, 128 * mp:128 * (mp + 1)],
                        vwT[:, kt, :], start=(kt == 0), stop=(kt == 1))
                nc.vector.tensor_copy(
                    vt1[:, mp, :, 0:DH],
                    ps[:, 0:C].rearrange("p (h d) -> p h d", h=NH))

        # ---------- MLP state (written per query-half) ----------
        pC = ctx.enter_context(tc.tile_pool(name="pC", bufs=1))
        x1u = pC.tile([128, 2, HW], FP16, tag="x1u")
        h1 = pC.tile([128, 8, H, WP], FP16, tag="h1")
        zpad = pC.tile([128, 8 * H * 2], F32, tag="zpad")
        nc.gpsimd.memset(zpad[:], 0.0)
        zsrc = zpad[:].rearrange("p (c a b) -> p c a b", c=8, a=H)
        nc.vector.tensor_copy(h1[:, :, :, 0:2], zsrc)
        nc.vector.tensor_copy(h1[:, :, :, WP - 2:WP], zsrc)
        hgr = pC.tile([128, 8, HW], FP16, tag="hgr")
        out_sb = pC.tile([128, 2, HW], F32, tag="out")
        h1f = h1[:].rearrange("p c a b -> p c (a b)")
        taps = [(dy, dx) for dy in (-1, 0, 1) for dx in (-1, 0, 1)]
        taps.remove((0, 0))
        taps = [(0, 0)] + taps      # center first: full coverage, start=True

        def mlp_emit(nh, mmpool, dwpools):
            """Emit the MLP of query-half nh as ~1-2us PE units (generator).

            fc1 rows 16nh..16nh+15 (= proj's n-half exactly); dw/gelu out
            rows 0-14 / 15-31 (one-row halo into the other half's fc1
            output); fc2 columns 480*nh-split.
            """
            n0 = NHALF * nh
            # proj + residual1 (columns n0:n0+512)
            for mt in range(2):
                pp = mmpool.tile([128, 512], F32, tag="mm", name=f"pj{nh}_{mt}")
                for kt in range(2):
                    nc.tensor.matmul(
                        pp[:], projT[:, kt, 128 * mt:128 * (mt + 1)],
                        attn_r[:, kt, n0:n0 + 512],
                        start=(kt == 0), stop=(kt == 1))
                nc.vector.scalar_tensor_tensor(
                    x1u[:, mt, n0:n0 + 512], pp[:], projb[:, mt:mt + 1],
                    x1[:, mt, n0:n0 + 512], ALU.add, ALU.add)
                yield
            # fc1: rows 16nh..16nh+15 = columns n0:n0+512
            for mt in range(8):
                pf = mmpool.tile([128, 512], F32, tag="mm", name=f"f1{nh}_{mt}")
                for kt in range(2):
                    nc.tensor.matmul(
                        pf[:], fc1T[:, kt, 128 * mt:128 * (mt + 1)],
                        x1u[:, kt, n0:n0 + 512],
                        start=(kt == 0), stop=(kt == 1))
                nc.vector.tensor_scalar_add(
                    h1[:, mt, 16 * nh:16 * nh + 16, 2:W + 2],
                    pf[:].rearrange("p (a b) -> p a b", a=16),
                    fc1b[:, mt:mt + 1])
                if mt % 2 == 1:
                    yield
            # depthwise 3x3 + tanh-gelu, out rows r0..r1-1
            r0, r1 = (0, 15) if nh == 0 else (15, 32)
            nrows = r1 - r0
            base = r0 * WP
            wl = 0 if nh == 0 else 14 * WP
            wh = 16 * WP if nh == 0 else PADF
            g0, gn = 32 * r0, 32 * nrows
            for ct in range(8):
                ps_dw = dwpools[ct % len(dwpools)].tile(
                    [128, 17 * WP], F32, tag="dw", name=f"dw{nh}_{ct}")
                for ti, (dy, dx) in enumerate(taps):
                    shift = dy * WP + dx
                    ws = 9 * ct + 3 * (dy + 1) + (dx + 1)
                    for (c0, n) in _dw_chunks(shift, base, wl, wh, nrows):
                        nc.tensor.matmul(
                            ps_dw[:, c0:c0 + n],
                            dwdt[:, ws, :],
                            h1f[:, ct, base + c0 + shift:base + c0 + shift + n],
                            start=(ti == 0), stop=(ti == len(taps) - 1))
                pin = ps_dw[:, 0:nrows * WP].rearrange(
                    "p (a b) -> p a b", a=nrows)[:, :, 2:W + 2]
                ut = pC.tile([128, 17 * W], F32, tag="ut")
                nc.scalar.activation(ut[:, 0:gn], pin, AF.Tanh, 0.0, GA)
                # hgr = (tanh + 1) * dw   (x0.5 folded into fc2 weights)
                nc.vector.scalar_tensor_tensor(
                    hgr[:, ct, g0:g0 + gn], ut[:, 0:gn], 1.0, pin,
                    ALU.add, ALU.mult)
                yield
            # fc2 + residual2 (columns g0:g0+gn)
            for mt in range(2):
                for cc0 in range(g0, g0 + gn, 512):
                    cn = min(512, g0 + gn - cc0)
                    pg = mmpool.tile([128, 512], F32, tag="mm",
                                  name=f"f2{nh}_{mt}_{cc0}")
                    for kt in range(8):
                        nc.tensor.matmul(
                            pg[:, 0:cn], fc2T[:, kt, 128 * mt:128 * (mt + 1)],
                            hgr[:, kt, cc0:cc0 + cn],
                            start=(kt == 0), stop=(kt == 7))
                    nc.vector.scalar_tensor_tensor(
                        out_sb[:, mt, cc0:cc0 + cn], pg[:, 0:cn],
                        fc2b[:, mt:mt + 1], x1u[:, mt, cc0:cc0 + cn],
                        ALU.add, ALU.add)
                    yield
            nc.sync.dma_start(outd[:, :, g0:g0 + gn], out_sb[:, :, g0:g0 + gn])

        # ---------- phase 2+3+4: attention halves with MLP overlap ----------
        filler = [None]
        dum_open = [True]

        def step():
            if filler[0] is not None:
                if next(filler[0], StopIteration) is StopIteration:
                    filler[0] = None
            elif dum_open[0]:
                warm(5)

        with tc.tile_pool(name="expS", bufs=6) as xpool, \
             tc.tile_pool(name="uz", bufs=3) as uz_pool, \
             tc.tile_pool(name="rtmp", bufs=4) as rpool, \
             tc.tile_pool(name="pss", bufs=2, space="PSUM") as pss, \
             tc.tile_pool(name="psa", bufs=1, space="PSUM") as psa:
            for nh in range(2):
                n0 = NHALF * nh
                for half in range(2):
                    for pr in range(2):
                        ja, jb = 2 * pr, 2 * pr + 1
                        heads = [(half * 4 + ja, 32 * ja, 0),
                                 (half * 4 + jb, 32 * jb, 64)]
                        ps_av = psa.tile([128, NHALF], F32, tag="av",
                                         name=f"av{nh}_{half}_{pr}")
                        pend = {h: None for h, _, _ in heads}
                        for mtp in range(4):
                            ps_s = {h: pss.tile([128, 2, NHALF], F32, tag="s",
                                                name=f"s{nh}_{h}_{mtp}")
                                    for h, _, _ in heads}
                            for h, hb, _ in heads:
                                for sub in range(2):
                                    mt = 2 * mtp + sub
                                    nc.tensor.matmul(
                                        ps_s[h][:, sub, :],
                                        k_sb[hb:hb + 32, half,
                                             128 * mt:128 * (mt + 1)],
                                        q_sb[hb:hb + 32, half, n0:n0 + NHALF],
                                        start=True, stop=True,
                                        tile_position=(hb, 0))
                            for h, hb, cb in heads:
                                ex = xpool.tile([128, 2, NHALF], BF16,
                                                tag="expS")
                                nc.scalar.activation(ex[:], ps_s[h][:], AF.Exp)
                                if pend[h] is not None:
                                    pmtp, pex = pend[h]
                                    for sub in range(2):
                                        nc.tensor.matmul(
                                            ps_av[cb:cb + DH + 1, :],
                                            vt1[:, 2 * pmtp + sub, h, :],
                                            pex[:, sub, :],
                                            start=(pmtp == 0 and sub == 0),
                                            stop=False,
                                            tile_position=(0, cb),
                                            skip_group_check=True)
                                pend[h] = (mtp, ex)
                            step()
                        for h, hb, cb in heads:
                            pmtp, pex = pend[h]
                            for sub in range(2):
                                nc.tensor.matmul(
                                    ps_av[cb:cb + DH + 1, :],
                                    vt1[:, 2 * pmtp + sub, h, :],
                                    pex[:, sub, :],
                                    start=False, stop=(sub == 1),
                                    tile_position=(0, cb),
                                    skip_group_check=True)
                        # evict + normalize per head (base-0 tiles: TT needs
                        # equal base partitions for both SBUF inputs)
                        for h, hb, cb in heads:
                            uz = uz_pool.tile([DH + 1, NHALF], F32, tag="uz")
                            nc.vector.tensor_copy(uz[:],
                                                  ps_av[cb:cb + DH + 1, :])
                            rt = rpool.tile([1, NHALF], F32, tag="rt")
                            nc.sync.dma_start(rt[:], uz[DH:DH + 1, :])
                            rr = rpool.tile([1, NHALF], F32, tag="rr")
                            nc.vector.reciprocal_approx_fast(rr[:], rt[:])
                            rb = rpool.tile([32, NHALF], F32, tag="rb")
                            nc.gpsimd.partition_broadcast(rb[:], rr[:])
                            nc.vector.tensor_mul(
                                attn_r[hb:hb + 32, half, n0:n0 + NHALF],
                                uz[0:DH, :], rb[:])
                        step()
                if nh == 0:
                    # close + consume the filler accumulation group (frees
                    # its PSUM bank for the nh=1 window: pss 4 + psa 1 +
                    # ps2 1 + psd 2 = 8 banks exactly)
                    nc.tensor.matmul(dum_ps[:], dW[:], dM[:],
                                     start=(ndum[0] == 0), stop=True,
                                     skip_group_check=True)
                    nc.vector.tensor_copy(sink[:], dum_ps[0:1, 0:4])
                    dum_open[0] = False
                    filler[0] = mlp_emit(0, ps2, [psd])
            while filler[0] is not None:
                step()

        with tc.tile_pool(name="mm2", bufs=3, space="PSUM") as ps2b, \
             tc.tile_pool(name="psd2", bufs=1, space="PSUM") as psd2:
            # second filler group: bridge the attention->mlp1 dependency
            # stall and keep the HAM clock warm through the tail
            dum2 = ps2b.tile([128, 512], F32, tag="mm", name="dum2")
            nd2 = 0
            for u, _ in enumerate(mlp_emit(1, ps2b, [psd, psd2])):
                if u < 10:
                    for _ in range(3):
                        nc.tensor.matmul(dum2[:], dW[:], dM[:],
                                         start=(nd2 == 0), stop=False,
                                         skip_group_check=True)
                        nd2 += 1
            nc.tensor.matmul(dum2[:], dW[:], dM[:],
                             start=False, stop=True, skip_group_check=True)
            nc.vector.tensor_copy(sink[:], dum2[0:1, 0:4])


def _build_nc():
    if "nc" in _NC_CACHE:
        return _NC_CACHE["nc"]
    nc = bacc.Bacc(trn_type="TRN2", target_bir_lowering=False, debug=False)
    io = {}
    for name, shape, dt in [
        ("x1", [128, 2, HW], FP16), ("x2", [128, 2, HW], FP16),
        ("qT", [128, 2, C], FP16), ("kT", [128, 2, C], FP16),
        ("vwT", [128, 2, C], FP16), ("projT", [128, 2, C], FP16),
        ("fc1T", [128, 2, HID], FP16), ("fc2T", [128, 8, C], FP16),
        ("dwd", [128, 72, 128], FP16),
        ("qb", [128, 2], F32), ("kb", [128, 2], F32), ("projb", [128, 2], F32),
        ("fc1b", [128, 8], F32), ("fc2b", [128, 2], F32),
        ("inv1", [128, 2], F32), ("beta1", [128, 2], F32),
    ]:
        io[name] = nc.dram_tensor(name, shape, dt, kind="ExternalInput").ap()
    io["out"] = nc.dram_tensor("out", [128, 2, HW], F32, kind="ExternalOutput").ap()

    with tile.TileContext(nc) as tc:
        _build_body(nc, tc, io)
    nc.compile()
    _NC_CACHE["nc"] = nc
    return nc


def _to_part_layout(a, ntiles):
    """[ntiles*128, F] -> [128, ntiles, F] with c = kt*128 + p."""
    return np.ascontiguousarray(
        a.reshape(ntiles, 128, -1).transpose(1, 0, 2))


def _bias_layout(b, ntiles):
    """[ntiles*128] -> [128, ntiles]."""
    return np.ascontiguousarray(b.reshape(ntiles, 128).T)


def _prepare_weights(bn1_g, bn1_b, bn1_m, bn1_v, q_w, k_w, v_w, temp, proj_w,
                     proj_b, bn2_g, bn2_b, bn2_m, bn2_v, fc1_w, fc1_b, dw_w,
                     fc2_w, fc2_b):
    f64 = np.float64
    inv1 = (bn1_g.astype(f64) / np.sqrt(bn1_v.astype(f64) + EPS))
    beta1 = bn1_b.astype(f64) - bn1_m.astype(f64) * inv1
    inv2 = (bn2_g.astype(f64) / np.sqrt(bn2_v.astype(f64) + EPS))
    beta2 = bn2_b.astype(f64) - bn2_m.astype(f64) * inv2

    tscale = np.repeat(temp.astype(f64), DH)                     # [256]
    qw_f = q_w.astype(f64) * inv1[None, :] * tscale[:, None]
    qb = (q_w.astype(f64) @ beta1) * tscale
    kw_f = k_w.astype(f64) * inv1[None, :]
    kb = k_w.astype(f64) @ beta1
    fc1w_f = fc1_w.astype(f64) * inv2[None, :]
    fc1bf = fc1_b.astype(f64) + fc1_w.astype(f64) @ beta2

    # [128, 72, 128]: partition-major diag blocks, (ct, tap) on free dim
    dwd = np.zeros((128, 72, 128), np.float32)
    idx = np.arange(128)
    for ct in range(8):
        for t in range(9):
            dy, dx = t // 3, t % 3
            dwd[idx, ct * 9 + t, idx] = dw_w[ct * 128 + idx, 0, dy, dx]

    f16 = np.float16
    w = {
        "qT": _to_part_layout(np.ascontiguousarray(qw_f.T).astype(f16), 2),
        "kT": _to_part_layout(np.ascontiguousarray(kw_f.T).astype(f16), 2),
        "vwT": _to_part_layout(np.ascontiguousarray(v_w.T).astype(f16), 2),
        "projT": _to_part_layout(np.ascontiguousarray(proj_w.T).astype(f16), 2),
        "fc1T": _to_part_layout(np.ascontiguousarray(fc1w_f.T).astype(f16), 2),
        # 0.5 of the tanh-gelu form folded into fc2
        "fc2T": _to_part_layout(
            np.ascontiguousarray(0.5 * fc2_w.astype(f64).T).astype(f16), 8),
        "dwd": dwd.astype(f16),
        "qb": _bias_layout(qb.astype(np.float32), 2),
        "kb": _bias_layout(kb.astype(np.float32), 2),
        "projb": _bias_layout(proj_b.astype(np.float32), 2),
        "fc1b": _bias_layout(fc1bf.astype(np.float32), 8),
        "fc2b": _bias_layout(fc2_b.astype(np.float32), 2),
        "inv1": _bias_layout(inv1.astype(np.float32), 2),
        "beta1": _bias_layout(beta1.astype(np.float32), 2),
    }
    return w


_LAST_RESULTS = {}


def kernel(x1, x2, bn1_g, bn1_b, bn1_m, bn1_v, q_w, k_w, v_w, temp, proj_w,
           proj_b, bn2_g, bn2_b, bn2_m, bn2_v, fc1_w, fc1_b, dw_w, fc2_w,
           fc2_b, _trace=False):
    x1 = np.asarray(x1, np.float32)
    x2 = np.asarray(x2, np.float32)
    args = [np.asarray(a) for a in
            (bn1_g, bn1_b, bn1_m, bn1_v, q_w, k_w, v_w, temp, proj_w, proj_b,
             bn2_g, bn2_b, bn2_m, bn2_v, fc1_w, fc1_b, dw_w, fc2_w, fc2_b)]
    w = _prepare_weights(*args)

    nc = _build_nc()
    in_maps = []
    for i in range(N_CORES):
        m = dict(w)
        m["x1"] = _to_part_layout(x1[i].reshape(C, HW), 2).astype(np.float16)
        m["x2"] = _to_part_layout(x2[i].reshape(C, HW), 2).astype(np.float16)
        in_maps.append(m)

    res = run_bass_kernel_spmd(nc, in_maps, core_ids=list(range(N_CORES)),
                               trace=_trace)
    _LAST_RESULTS["res"] = res

    out = np.empty((B, C, H, W), np.float32)
    for i in range(N_CORES):
        o = res.results[i]["out"]                    # [128, 2, 1024]
        out[i] = o.transpose(1, 0, 2).reshape(C, H, W)
    return out


# revision 27
# speedup vs baseline: 1.1960x; 1.0028x over previous
"""CMXBlock (dense transformer block) Trainium2 Bass kernel.

Sharding: data-parallel over batch B=8 across the 8 NeuronCores — one image
per core, all weights replicated, no collectives.

Per-core computation (C=256 channels on partitions, HW=1024 positions free):
  x1 <- x1 + proj(softmax((q_w@bn1(x1))^T (k_w@bn1(x2)) * temp) @ (v_w@bn1(x2))^T)
  x1 <- x1 + fc2(gelu(dwconv3x3(fc1(bn2(x1)))))

Implementation notes (v4):
 - All matmul operands are 16-bit: fp16 for activations/weights, bf16 for
   the softmax numerators (range up to e^~30 overflows fp16).  16-bit
   stationary operands enable FWL; 16-bit moving operands let ACT/DVE feed
   the PE directly.
 - BatchNorms/temp folded into the 1x1-conv weights host-side.
 - Softmax: transposed scores S^T[m,n], 2 heads row-tiled (K=32 at row
   groups hb, hb+32), exp on ACT straight to bf16, denominator via an
   all-ones column in the AV stationary, both heads' AV col-tiled into one
   shared PSUM tile (ja at 0:33 / col groups 0-1, jb at 64:97 / 2-3).
 - The image is processed in two query/pixel halves: attention(half 1)'s
   ACT-bound softmax overlaps the MLP of half 0 on the PE (the MLP emission
   is a generator stepped from inside the attention loop).  During
   attention(half 0) zero-operand filler matmuls keep the PE HAM clock at
   K=8/8.
 - gelu is computed as x*0.5*(1+tanh(0.886x)) (max abs err 0.014): tanh
   lives in the same ACT table set as exp, so the interleaved softmax/MLP
   phases never thrash the 2.7us ACT table load.  The 0.5 and the (1+tanh)
   combine are folded into the fc2 weights / one DVE op.
 - Depthwise 3x3 runs on the PE as 9 diagonal-matrix taps over an x-padded
   [32, 36] spatial layout; all 72 diagonal tap tiles are preloaded in one
   DMA (per-tap DMA streaming was the MLP bottleneck).
 - MLP row split: fc1 rows 0-15 / 16-31 (aligns with proj's n-halves), dw
   and gelu rows 0-14 / 15-31 (one-row halo), fc2 columns 0-480 / 480-1024.
"""
import numpy as np

import concourse.bass as bass
import concourse.tile as tile
import concourse.mybir as mybir
from concourse import bacc
from concourse.bass_utils import run_bass_kernel_spmd

import concourse.bass_utils as _bu

if not getattr(_bu, "_ldwopt_patched", False):
    _orig_run_command = _bu.run_command

    def _run_command_ldwopt(cmd, **kw):
        cmd = list(cmd)  # ldw-opt=true fails walrus visitInstLdweights codegen
        return _orig_run_command(cmd, **kw)

    _bu.run_command = _run_command_ldwopt
    _bu._ldwopt_patched = True

F32 = mybir.dt.float32
BF16 = mybir.dt.bfloat16
FP16 = mybir.dt.float16
AF = mybir.ActivationFunctionType
ALU = mybir.AluOpType

B, C, H, W = 8, 256, 32, 32
NH, DH = 8, 32          # heads, head dim
HW = H * W              # 1024 positions
HID = 4 * C             # 1024 mlp hidden channels
EPS = 1e-5
WP = W + 4              # x-padded row width (36, even)
PADF = H * WP           # padded flat spatial size (1152)
N_CORES = 8
GA = 0.886              # tanh-gelu constant
NHALF = 512             # positions per query half

_NC_CACHE = {}


def _dw_chunks(shift, base, wl, wh, nrows):
    """Even-aligned bank chunks (<=512) of a dw tap's local dst range.

    Local dst f in [0, nrows*36); full-flat src = base + f + shift must lie
    in the written h1 window [wl, wh).  Elements dropped by even-alignment
    are x-pad columns (never read downstream) covered by the center tap.
    """
    n = nrows * WP
    lo = max(0, wl - base - shift)
    hi = min(n, wh - base - shift)
    out = []
    for b0 in range(0, n, 512):
        a, b = max(lo, b0), min(hi, b0 + 512)
        a += a % 2
        m = (b - a) & ~1
        if m > 0:
            out.append((a, m))
    return out


def _build_body(nc, tc, io):
    x1d, x2d = io["x1"], io["x2"]
    outd = io["out"]

    import contextlib
    ctx = contextlib.ExitStack()
    with ctx:
        wpool = ctx.enter_context(tc.tile_pool(name="weights", bufs=1))
        pB = ctx.enter_context(tc.tile_pool(name="pB", bufs=1))

        # ---------- persistent SBUF tensors ----------
        # DMA order matters: the first compute (q = qT @ x1) can start once
        # x1+qT land; the big late-use tensors (fc*, dwd) stream in last,
        # overlapped under attention.
        x1 = wpool.tile([128, 2, HW], FP16, tag="x1")
        nc.sync.dma_start(x1[:], x1d[:])

        def wload(name, shape, dt):
            t = wpool.tile(shape, dt, tag=name)
            nc.sync.dma_start(t[:], io[name][:])
            return t

        qT = wload("qT", [128, 2, C], FP16)
        qb = wload("qb", [128, 2], F32)
        x2 = wpool.tile([128, 2, HW], FP16, tag="x2")
        nc.sync.dma_start(x2[:], x2d[:])
        kT = wload("kT", [128, 2, C], FP16)
        kb = wload("kb", [128, 2], F32)
        inv1 = wload("inv1", [128, 2], F32)
        beta1 = wload("beta1", [128, 2], F32)
        vwT = wload("vwT", [128, 2, C], FP16)
        projT = wload("projT", [128, 2, C], FP16)
        projb = wload("projb", [128, 2], F32)
        fc1T = wload("fc1T", [128, 2, HID], FP16)
        fc1b = wload("fc1b", [128, 8], F32)
        fc2T = wload("fc2T", [128, 8, C], FP16)
        fc2b = wload("fc2b", [128, 2], F32)
        dwdt = wload("dwd", [128, 72, 128], FP16)   # 8ct x 9taps diag blocks

        attn_r = pB.tile([128, 2, HW], FP16, tag="attn_r")
        # PE-warmth filler operands (zeros; keep HAM at K=8/8 through the
        # ACT-bound softmax of query-half 0)
        dW = wpool.tile([128, 128], FP16, tag="dW")
        dM = wpool.tile([128, 512], FP16, tag="dM")
        sink = wpool.tile([1, 4], F32, tag="sink")
        nc.gpsimd.memset(dW[:], 0.0)
        nc.gpsimd.memset(dM[:], 0.0)

        ps2 = ctx.enter_context(tc.tile_pool(name="ps2", bufs=1, space="PSUM"))
        psd = ctx.enter_context(tc.tile_pool(name="psd", bufs=1, space="PSUM"))
        # filler accumulator borrows the (idle until mlp0) ps2 slot
        dum_ps = ps2.tile([128, 512], F32, tag="mm", name="dum")
        ndum = [0]

        def warm(n):
            for _ in range(n):
                nc.tensor.matmul(dum_ps[:], dW[:], dM[:],
                                 start=(ndum[0] == 0), stop=False,
                                 skip_group_check=True)
                ndum[0] += 1

        warm(16)   # keep the PE HAM clock spinning during the input DMAs

        pA = ctx.enter_context(tc.tile_pool(name="pA", bufs=1))
        q_sb = pA.tile([128, 2, HW], FP16, tag="q")
        k_sb = pA.tile([128, 2, HW], FP16, tag="k")
        vt1 = pA.tile([128, 8, NH, DH + 1], BF16, tag="vt1")  # [p, mt, h, v|1]
        onesf = pA.tile([128, 8 * NH], F32, tag="onesf")
        nc.gpsimd.memset(onesf[:], 1.0)
        nc.vector.tensor_copy(
            vt1[:, :, :, DH:DH + 1],
            onesf[:].rearrange("p (a b c) -> p a b c", a=8, b=NH))

        # ---------- phase 1: q, k projections; x2n; v^T ----------
        with tc.tile_pool(name="p1", bufs=1) as p1, \
             tc.tile_pool(name="ps1", bufs=4, space="PSUM") as ps1:
            for (wT, bias, dst) in ((qT, qb, q_sb), (kT, kb, k_sb)):
                rhs = x1 if dst is q_sb else x2
                for mt in range(2):
                    pss2 = [ps1.tile([128, 512], F32, tag="mm",
                                     name=f"qk_{mt}_{chk}") for chk in range(2)]
                    for kt in range(2):
                        for chk in range(2):
                            nc.tensor.matmul(
                                pss2[chk][:], wT[:, kt, 128 * mt:128 * (mt + 1)],
                                rhs[:, kt, 512 * chk:512 * (chk + 1)],
                                start=(kt == 0), stop=(kt == 1))
                    for chk in range(2):
                        nc.vector.tensor_scalar_add(
                            dst[:, mt, 512 * chk:512 * (chk + 1)], pss2[chk][:],
                            bias[:, mt:mt + 1])

            x2n = p1.tile([128, 2, HW], FP16, tag="x2n")
            for kt in range(2):
                nc.vector.tensor_scalar(
                    x2n[:, kt, :], x2[:, kt, :],
                    inv1[:, kt:kt + 1], beta1[:, kt:kt + 1], ALU.mult, ALU.add)

            for mp in range(8):
                ps = ps1.tile([128, 512], F32, tag="mm")
                for kt in range(2):
                    nc.tensor.matmul(
                        ps[:, 0:C], x2n[:, kt, 128 * mp:128 * (mp + 1)],
                        vwT[:, kt, :], start=(kt == 0), stop=(kt == 1))
                nc.vector.tensor_copy(
                    vt1[:, mp, :, 0:DH],
                    ps[:, 0:C].rearrange("p (h d) -> p h d", h=NH))

        # ---------- MLP state (written per query-half) ----------
        pC = ctx.enter_context(tc.tile_pool(name="pC", bufs=1))
        x1u = pC.tile([128, 2, HW], FP16, tag="x1u")
        h1 = pC.tile([128, 8, H, WP], FP16, tag="h1")
        zpad = pC.tile([128, 8 * H * 2], F32, tag="zpad")
        nc.gpsimd.memset(zpad[:], 0.0)
        zsrc = zpad[:].rearrange("p (c a b) -> p c a b", c=8, a=H)
        nc.vector.tensor_copy(h1[:, :, :, 0:2], zsrc)
        nc.vector.tensor_copy(h1[:, :, :, WP - 2:WP], zsrc)
        hgr = pC.tile([128, 8, HW], FP16, tag="hgr")
        out_sb = pC.tile([128, 2, HW], F32, tag="out")
        h1f = h1[:].rearrange("p c a b -> p c (a b)")
        taps = [(dy, dx) for dy in (-1, 0, 1) for dx in (-1, 0, 1)]
        taps.remove((0, 0))
        taps = [(0, 0)] + taps      # center first: full coverage, start=True

        def mlp_emit(nh, mmpool, dwpools):
            """Emit the MLP of query-half nh as ~1-2us PE units (generator).

            fc1 rows 16nh..16nh+15 (= proj's n-half exactly); dw/gelu out
            rows 0-14 / 15-31 (one-row halo into the other half's fc1
            output); fc2 columns 480*nh-split.
            """
            n0 = NHALF * nh
            # proj + residual1 (columns n0:n0+512)
            for mt in range(2):
                pp = mmpool.tile([128, 512], F32, tag="mm", name=f"pj{nh}_{mt}")
                for kt in range(2):
                    nc.tensor.matmul(
                        pp[:], projT[:, kt, 128 * mt:128 * (mt + 1)],
                        attn_r[:, kt, n0:n0 + 512],
                        start=(kt == 0), stop=(kt == 1))
                nc.vector.scalar_tensor_tensor(
                    x1u[:, mt, n0:n0 + 512], pp[:], projb[:, mt:mt + 1],
                    x1[:, mt, n0:n0 + 512], ALU.add, ALU.add)
                yield
            # fc1: rows 16nh..16nh+15 = columns n0:n0+512
            for mt in range(8):
                pf = mmpool.tile([128, 512], F32, tag="mm", name=f"f1{nh}_{mt}")
                for kt in range(2):
                    nc.tensor.matmul(
                        pf[:], fc1T[:, kt, 128 * mt:128 * (mt + 1)],
                        x1u[:, kt, n0:n0 + 512],
                        start=(kt == 0), stop=(kt == 1))
                nc.vector.tensor_scalar_add(
                    h1[:, mt, 16 * nh:16 * nh + 16, 2:W + 2],
                    pf[:].rearrange("p (a b) -> p a b", a=16),
                    fc1b[:, mt:mt + 1])
                if mt % 2 == 1:
                    yield
            # depthwise 3x3 + tanh-gelu, out rows r0..r1-1
            r0, r1 = (0, 15) if nh == 0 else (15, 32)
            nrows = r1 - r0
            base = r0 * WP
            wl = 0 if nh == 0 else 14 * WP
            wh = 16 * WP if nh == 0 else PADF
            g0, gn = 32 * r0, 32 * nrows
            for ct in range(8):
                ps_dw = dwpools[ct % len(dwpools)].tile(
                    [128, 17 * WP], F32, tag="dw", name=f"dw{nh}_{ct}")
                for ti, (dy, dx) in enumerate(taps):
                    shift = dy * WP + dx
                    ws = 9 * ct + 3 * (dy + 1) + (dx + 1)
                    for (c0, n) in _dw_chunks(shift, base, wl, wh, nrows):
                        nc.tensor.matmul(
                            ps_dw[:, c0:c0 + n],
                            dwdt[:, ws, :],
                            h1f[:, ct, base + c0 + shift:base + c0 + shift + n],
                            start=(ti == 0), stop=(ti == len(taps) - 1))
                pin = ps_dw[:, 0:nrows * WP].rearrange(
                    "p (a b) -> p a b", a=nrows)[:, :, 2:W + 2]
                ut = pC.tile([128, 17 * W], F32, tag="ut")
                nc.scalar.activation(ut[:, 0:gn], pin, AF.Tanh, 0.0, GA)
                # hgr = (tanh + 1) * dw   (x0.5 folded into fc2 weights)
                nc.vector.scalar_tensor_tensor(
                    hgr[:, ct, g0:g0 + gn], ut[:, 0:gn], 1.0, pin,
                    ALU.add, ALU.mult)
                yield
            # fc2 + residual2 (columns g0:g0+gn)
            for mt in range(2):
                for cc0 in range(g0, g0 + gn, 512):
                    cn = min(512, g0 + gn - cc0)
                    pg = mmpool.tile([128, 512], F32, tag="mm",
                                  name=f"f2{nh}_{mt}_{cc0}")
                    for kt in range(8):
                        nc.tensor.matmul(
                            pg[:, 0:cn], fc2T[:, kt, 128 * mt:128 * (mt + 1)],
                            hgr[:, kt, cc0:cc0 + cn],
                            start=(kt == 0), stop=(kt == 7))
                    nc.vector.scalar_tensor_tensor(
                        out_sb[:, mt, cc0:cc0 + cn], pg[:, 0:cn],
                        fc2b[:, mt:mt + 1], x1u[:, mt, cc0:cc0 + cn],
                        ALU.add, ALU.add)
                    yield
            nc.sync.dma_start(outd[:, :, g0:g0 + gn], out_sb[:, :, g0:g0 + gn])

        # ---------- phase 2+3+4: attention halves with MLP overlap ----------
        filler = [None]
        dum_open = [True]

        def step():
            if filler[0] is not None:
                if next(filler[0], StopIteration) is StopIteration:
                    filler[0] = None
            elif dum_open[0]:
                warm(4)

        with tc.tile_pool(name="expS", bufs=6) as xpool, \
             tc.tile_pool(name="uz", bufs=3) as uz_pool, \
             tc.tile_pool(name="rtmp", bufs=4) as rpool, \
             tc.tile_pool(name="pss", bufs=2, space="PSUM") as pss, \
             tc.tile_pool(name="psa", bufs=1, space="PSUM") as psa:
            for nh in range(2):
                n0 = NHALF * nh
                for half in range(2):
                    for pr in range(2):
                        ja, jb = 2 * pr, 2 * pr + 1
                        heads = [(half * 4 + ja, 32 * ja, 0),
                                 (half * 4 + jb, 32 * jb, 64)]
                        ps_av = psa.tile([128, NHALF], F32, tag="av",
                                         name=f"av{nh}_{half}_{pr}")
                        pend = {h: None for h, _, _ in heads}
                        for mtp in range(4):
                            ps_s = {h: pss.tile([128, 2, NHALF], F32, tag="s",
                                                name=f"s{nh}_{h}_{mtp}")
                                    for h, _, _ in heads}
                            for h, hb, _ in heads:
                                for sub in range(2):
                                    mt = 2 * mtp + sub
                                    nc.tensor.matmul(
                                        ps_s[h][:, sub, :],
                                        k_sb[hb:hb + 32, half,
                                             128 * mt:128 * (mt + 1)],
                                        q_sb[hb:hb + 32, half, n0:n0 + NHALF],
                                        start=True, stop=True,
                                        tile_position=(hb, 0))
                            for h, hb, cb in heads:
                                ex = xpool.tile([128, 2, NHALF], BF16,
                                                tag="expS")
                                nc.scalar.activation(ex[:], ps_s[h][:], AF.Exp)
                                if pend[h] is not None:
                                    pmtp, pex = pend[h]
                                    for sub in range(2):
                                        nc.tensor.matmul(
                                            ps_av[cb:cb + DH + 1, :],
                                            vt1[:, 2 * pmtp + sub, h, :],
                                            pex[:, sub, :],
                                            start=(pmtp == 0 and sub == 0),
                                            stop=False,
                                            tile_position=(0, cb),
                                            skip_group_check=True)
                                pend[h] = (mtp, ex)
                            step()
                        for h, hb, cb in heads:
                            pmtp, pex = pend[h]
                            for sub in range(2):
                                nc.tensor.matmul(
                                    ps_av[cb:cb + DH + 1, :],
                                    vt1[:, 2 * pmtp + sub, h, :],
                                    pex[:, sub, :],
                                    start=False, stop=(sub == 1),
                                    tile_position=(0, cb),
                                    skip_group_check=True)
                        # evict + normalize per head (base-0 tiles: TT needs
                        # equal base partitions for both SBUF inputs)
                        for h, hb, cb in heads:
                            uz = uz_pool.tile([DH + 1, NHALF], F32, tag="uz")
                            nc.vector.tensor_copy(uz[:],
                                                  ps_av[cb:cb + DH + 1, :])
                            rt = rpool.tile([1, NHALF], F32, tag="rt")
                            nc.sync.dma_start(rt[:], uz[DH:DH + 1, :])
                            rr = rpool.tile([1, NHALF], F32, tag="rr")
                            nc.vector.reciprocal_approx_fast(rr[:], rt[:])
                            rb = rpool.tile([32, NHALF], F32, tag="rb")
                            nc.gpsimd.partition_broadcast(rb[:], rr[:])
                            nc.vector.tensor_mul(
                                attn_r[hb:hb + 32, half, n0:n0 + NHALF],
                                uz[0:DH, :], rb[:])
                        step()
                if nh == 0:
                    # close + consume the filler accumulation group (frees
                    # its PSUM bank for the nh=1 window: pss 4 + psa 1 +
                    # ps2 1 + psd 2 = 8 banks exactly)
                    nc.tensor.matmul(dum_ps[:], dW[:], dM[:],
                                     start=(ndum[0] == 0), stop=True,
                                     skip_group_check=True)
                    nc.vector.tensor_copy(sink[:], dum_ps[0:1, 0:4])
                    dum_open[0] = False
                    filler[0] = mlp_emit(0, ps2, [psd])
            while filler[0] is not None:
                step()

        with tc.tile_pool(name="mm2", bufs=3, space="PSUM") as ps2b, \
             tc.tile_pool(name="psd2", bufs=1, space="PSUM") as psd2:
            # second filler group: bridge the attention->mlp1 dependency
            # stall and keep the HAM clock warm through the tail
            dum2 = ps2b.tile([128, 512], F32, tag="mm", name="dum2")
            nd2 = 0
            for u, _ in enumerate(mlp_emit(1, ps2b, [psd, psd2])):
                if u < 10:
                    for _ in range(2):
                        nc.tensor.matmul(dum2[:], dW[:], dM[:],
                                         start=(nd2 == 0), stop=False,
                                         skip_group_check=True)
                        nd2 += 1
            nc.tensor.matmul(dum2[:], dW[:], dM[:],
                             start=False, stop=True, skip_group_check=True)
            nc.vector.tensor_copy(sink[:], dum2[0:1, 0:4])


def _build_nc():
    if "nc" in _NC_CACHE:
        return _NC_CACHE["nc"]
    nc = bacc.Bacc(trn_type="TRN2", target_bir_lowering=False, debug=False)
    io = {}
    for name, shape, dt in [
        ("x1", [128, 2, HW], FP16), ("x2", [128, 2, HW], FP16),
        ("qT", [128, 2, C], FP16), ("kT", [128, 2, C], FP16),
        ("vwT", [128, 2, C], FP16), ("projT", [128, 2, C], FP16),
        ("fc1T", [128, 2, HID], FP16), ("fc2T", [128, 8, C], FP16),
        ("dwd", [128, 72, 128], FP16),
        ("qb", [128, 2], F32), ("kb", [128, 2], F32), ("projb", [128, 2], F32),
        ("fc1b", [128, 8], F32), ("fc2b", [128, 2], F32),
        ("inv1", [128, 2], F32), ("beta1", [128, 2], F32),
    ]:
        io[name] = nc.dram_tensor(name, shape, dt, kind="ExternalInput").ap()
    io["out"] = nc.dram_tensor("out", [128, 2, HW], F32, kind="ExternalOutput").ap()

    with tile.TileContext(nc) as tc:
        _build_body(nc, tc, io)
    nc.compile()
    _NC_CACHE["nc"] = nc
    return nc


def _to_part_layout(a, ntiles):
    """[ntiles*128, F] -> [128, ntiles, F] with c = kt*128 + p."""
    return np.ascontiguousarray(
        a.reshape(ntiles, 128, -1).transpose(1, 0, 2))


def _bias_layout(b, ntiles):
    """[ntiles*128] -> [128, ntiles]."""
    return np.ascontiguousarray(b.reshape(ntiles, 128).T)


def _prepare_weights(bn1_g, bn1_b, bn1_m, bn1_v, q_w, k_w, v_w, temp, proj_w,
                     proj_b, bn2_g, bn2_b, bn2_m, bn2_v, fc1_w, fc1_b, dw_w,
                     fc2_w, fc2_b):
    f64 = np.float64
    inv1 = (bn1_g.astype(f64) / np.sqrt(bn1_v.astype(f64) + EPS))
    beta1 = bn1_b.astype(f64) - bn1_m.astype(f64) * inv1
    inv2 = (bn2_g.astype(f64) / np.sqrt(bn2_v.astype(f64) + EPS))
    beta2 = bn2_b.astype(f64) - bn2_m.astype(f64) * inv2

    tscale = np.repeat(temp.astype(f64), DH)                     # [256]
    qw_f = q_w.astype(f64) * inv1[None, :] * tscale[:, None]
    qb = (q_w.astype(f64) @ beta1) * tscale
    kw_f = k_w.astype(f64) * inv1[None, :]
    kb = k_w.astype(f64) @ beta1
    fc1w_f = fc1_w.astype(f64) * inv2[None, :]
    fc1bf = fc1_b.astype(f64) + fc1_w.astype(f64) @ beta2

    # [128, 72, 128]: partition-major diag blocks, (ct, tap) on free dim
    dwd = np.zeros((128, 72, 128), np.float32)
    idx = np.arange(128)
    for ct in range(8):
        for t in range(9):
            dy, dx = t // 3, t % 3
            dwd[idx, ct * 9 + t, idx] = dw_w[ct * 128 + idx, 0, dy, dx]

    f16 = np.float16
    w = {
        "qT": _to_part_layout(np.ascontiguousarray(qw_f.T).astype(f16), 2),
        "kT": _to_part_layout(np.ascontiguousarray(kw_f.T).astype(f16), 2),
        "vwT": _to_part_layout(np.ascontiguousarray(v_w.T).astype(f16), 2),
        "projT": _to_part_layout(np.ascontiguousarray(proj_w.T).astype(f16), 2),
        "fc1T": _to_part_layout(np.ascontiguousarray(fc1w_f.T).astype(f16), 2),
        # 0.5 of the tanh-gelu form folded into fc2
        "fc2T": _to_part_layout(
            np.ascontiguousarray(0.5 * fc2_w.astype(f64).T).astype(f16), 8),
        "dwd": dwd.astype(f16),
        "qb": _bias_layout(qb.astype(np.float32), 2),
        "kb": _bias_layout(kb.astype(np.float32), 2),
        "projb": _bias_layout(proj_b.astype(np.float32), 2),
        "fc1b": _bias_layout(fc1bf.astype(np.float32), 8),
        "fc2b": _bias_layout(fc2_b.astype(np.float32), 2),
        "inv1": _bias_layout(inv1.astype(np.float32), 2),
        "beta1": _bias_layout(beta1.astype(np.float32), 2),
    }
    return w


_LAST_RESULTS = {}


def kernel(x1, x2, bn1_g, bn1_b, bn1_m, bn1_v, q_w, k_w, v_w, temp, proj_w,
           proj_b, bn2_g, bn2_b, bn2_m, bn2_v, fc1_w, fc1_b, dw_w, fc2_w,
           fc2_b, _trace=False):
    x1 = np.asarray(x1, np.float32)
    x2 = np.asarray(x2, np.float32)
    args = [np.asarray(a) for a in
            (bn1_g, bn1_b, bn1_m, bn1_v, q_w, k_w, v_w, temp, proj_w, proj_b,
             bn2_g, bn2_b, bn2_m, bn2_v, fc1_w, fc1_b, dw_w, fc2_w, fc2_b)]
    w = _prepare_weights(*args)

    nc = _build_nc()
    in_maps = []
    for i in range(N_CORES):
        m = dict(w)
        m["x1"] = _to_part_layout(x1[i].reshape(C, HW), 2).astype(np.float16)
        m["x2"] = _to_part_layout(x2[i].reshape(C, HW), 2).astype(np.float16)
        in_maps.append(m)

    res = run_bass_kernel_spmd(nc, in_maps, core_ids=list(range(N_CORES)),
                               trace=_trace)
    _LAST_RESULTS["res"] = res

    out = np.empty((B, C, H, W), np.float32)
    for i in range(N_CORES):
        o = res.results[i]["out"]                    # [128, 2, 1024]
        out[i] = o.transpose(1, 0, 2).reshape(C, H, W)
    return out
